# revision 1
# baseline (speedup 1.0000x reference)
"""Trainium2 Bass kernel for nn_DecoupledCls (RAB transformer + conv head).

Sharding: 8 cores = (batch b, sequence half). Core (b, 0) owns tokens
[0, 1152); core (b, 1) owns tokens [1096, 2248) of the S = 2248 token
sequence (T=2048 video tokens + C=200 prototype tokens). All per-token
ops run on local tokens; K/V are computed redundantly over the full S
from a replicated "2-block" feature-major src buffer. One pairwise
AllGather exchanges updated src halves between layers 1 and 2.

Host-side folding: the query/key 1x1 convs are fused into the MHA
in-projections (wQC = wQ_mha @ wq_conv etc., folded in fp64), so each
layer runs: K/V/Q projections, flash-style attention (S^T layout,
exp-sum via ones-matmul), out-proj + residual + LN (stats via
ones-matmul over the feature-partition axis), FFN + residual + LN.
Final Conv1d(k=3) runs as 3 shifted matmuls with a data-driven mask
that zeroes prototype tokens at the conv input.

Layout conventions:
  - activations feature-major: (F partitions, tokens free), F = 8 tiles
  - attention per head: K^T/Q^T (dh=128, keys/queries), V token-major
  - matmuls: fp32r for residual-critical paths, bf16 for attention/conv
  - weight DMAs ride the gpsimd SWDGE ring (ACT-ring DMAs + collectives
    hang NRT); streaming DMAs use the SP HWDGE ring
"""
import math
import numpy as np

import concourse.bacc as bacc
import concourse.mybir as mybir
import concourse.tile as tile
from concourse.bass_utils import run_bass_kernel_spmd

dt = mybir.dt
AF = mybir.ActivationFunctionType
OP = mybir.AluOpType

L, B, T, F, C, H, DFF = 2, 4, 2048, 1024, 200, 8, 128
S = T + C                  # 2248
NLOC = 1152                # local tokens per core (9*128)
R1 = S - NLOC              # 1096: start of half-1 local range
CONV_SPLIT = 1120          # conv output ownership split
SQB = 384                  # query-block width (3 blocks cover NLOC)
NSQ = NLOC // SQB
NF = F // 128              # 8 feature tiles
SCALE = float(1.0 / math.sqrt(128.0))
REPLICA_GROUPS = [[0, 1], [2, 3], [4, 5], [6, 7]]

# sk blocks: (block, col0, width, tile0). Enumerates all 2248 keys once:
# block 0 = tokens [0,1152), block 1 cols [56,1152) = tokens [1152,2248).
SK_BLOCKS = [
    (0, 0, 512, 0), (0, 512, 512, 4), (0, 1024, 128, 8),
    (1, 56, 512, 9), (1, 568, 512, 13), (1, 1080, 72, 17),
]
# per-tile list: (block, col0, width, tile_idx)
SK_TILES = []
for _blk, _c0, _w, _t0 in SK_BLOCKS:
    _off = 0
    while _off < _w:
        _wt = min(128, _w - _off)
        SK_TILES.append((_blk, _c0 + _off, _wt, _t0 + _off // 128))
        _off += _wt
NT = len(SK_TILES)         # 18

_CACHE = {}


def _build_program(no_cc=False):
    key = ("nc", no_cc)
    if key in _CACHE:
        return _CACHE[key]

    nc = bacc.Bacc("TRN2", target_bir_lowering=False, debug=False,
                   num_devices=8)

    def din(name, shape, dty=dt.float32):
        return nc.dram_tensor(name, list(shape), dty, kind="ExternalInput")

    srcfull = din("srcfull", (2, F, NLOC), dt.bfloat16)
    srcloc = din("srcloc", (F, NLOC))
    maskin = din("maskin", (1, NLOC), dt.bfloat16)
    wkc = din("wkc", (L, F, F), dt.bfloat16)
    wv = din("wv", (L, F, F), dt.bfloat16)
    wqc = din("wqc", (L, F, F), dt.bfloat16)
    wo = din("wo", (L, F, F), dt.bfloat16)
    wl1 = din("wl1", (L, F, DFF), dt.bfloat16)
    wl2 = din("wl2", (L, DFF, F), dt.bfloat16)
    wemb = din("wemb", (3, F, F), dt.bfloat16)
    bkc = din("bkc", (L, F, 1))
    bqc = din("bqc", (L, F, 1))
    bo = din("bo", (L, F, 1))
    bl1 = din("bl1", (L, DFF, 1))
    bl2 = din("bl2", (L, F, 1))
    g1 = din("g1", (L, F, 1))
    be1 = din("be1", (L, F, 1))
    g2 = din("g2", (L, F, 1))
    be2 = din("be2", (L, F, 1))
    bemb = din("bemb", (F, 1))
    out = nc.dram_tensor("out", [F, NLOC], dt.float32, kind="ExternalOutput")

    with tile.TileContext(nc) as tc:
        _emit(nc, tc, locals(), no_cc=no_cc)
    nc.compile()
    _CACHE[key] = nc
    return nc


def _emit(nc, tc, t, no_cc=False):
    from contextlib import ExitStack
    srcfull, srcloc, maskin = t["srcfull"], t["srcloc"], t["maskin"]
    wkc, wv, wqc, wo, wl1, wl2, wemb = (t["wkc"], t["wv"], t["wqc"], t["wo"],
                                        t["wl1"], t["wl2"], t["wemb"])
    bkc, bqc, bo, bl1, bl2 = (t["bkc"], t["bqc"], t["bo"], t["bl1"],
                              t["bl2"])
    g1, be1, g2, be2, bemb, out = (t["g1"], t["be1"], t["g2"], t["be2"],
                                   t["bemb"], t["out"])

    ctx = ExitStack()
    with ctx:
        # ---------------- pools ----------------
        # left-side: cn/bias/res/a1w permanent; kv + phase pools per layer.
        # right-side: a2w permanent; bc (O8 + C-phase weights) per layer.
        cn = ctx.enter_context(tc.tile_pool(name="cn", bufs=1))
        bias = ctx.enter_context(tc.tile_pool(name="bias", bufs=1))
        res = ctx.enter_context(tc.tile_pool(name="res", bufs=1))
        dram = ctx.enter_context(tc.tile_pool(name="dram", bufs=1,
                                              space="DRAM"))

        # ---------------- constants ----------------
        ones_bf = cn.tile([128, 1], dt.bfloat16)
        nc.gpsimd.memset(ones_bf[:], 1.0)
        ones_f = cn.tile([128, 1], dt.float32)
        nc.gpsimd.memset(ones_f[:], 1.0)
        ones_r = cn.tile([128, 1], dt.float32r)
        nc.scalar.copy(ones_r[:], ones_f[:])
        eps_t = cn.tile([1, 1], dt.float32)
        nc.gpsimd.memset(eps_t[:], 1e-5)

        # ---------------- resident activations ----------------
        # SRCB has a zero column on each side so the conv taps never run
        # off the end; all data accesses are shifted by +1.
        SRC = [res.tile([128, NLOC], dt.float32r, tag=f"src{i}",
                        name=f"src{i}") for i in range(NF)]
        SRCB = [res.tile([128, NLOC + 2], dt.bfloat16, tag=f"srcb{i}",
                         name=f"srcb{i}") for i in range(NF)]

        def init_src():
            for i in range(NF):
                eng = nc.gpsimd if i % 2 == 0 else nc.sync
                eng.dma_start(
                    SRC[i][:],
                    srcloc[i * 128:(i + 1) * 128, :].bitcast(dt.float32r))
                nc.scalar.copy(SRCB[i][:, 1:NLOC + 1], SRC[i][:])
                nc.gpsimd.memset(SRCB[i][:, 0:1], 0.0)
                nc.gpsimd.memset(SRCB[i][:, NLOC + 1:NLOC + 2], 0.0)

        # conv-input mask (zeroes prototype tokens on the half-1 cores)
        mrow = res.tile([1, NLOC], dt.bfloat16, tag="mrow", name="mrow")
        nc.sync.dma_start(mrow[:], maskin[:])
        mbc = res.tile([128, NLOC], dt.bfloat16, tag="mbc", name="mbc")
        nc.gpsimd.partition_broadcast(mbc[:], mrow[:])

        gin = dram.tile([F, NLOC], dt.bfloat16)
        gout = dram.tile([2, F, NLOC], dt.bfloat16)

        def ldbias(pool_tag, src_ap):
            b = bias.tile([src_ap.shape[0], 1], dt.float32, tag=pool_tag,
                          name=pool_tag)
            nc.sync.dma_start(b[:], src_ap)
            return b

        for layer in range(L):
            es_kv = ExitStack()
            es_bc = ExitStack()
            kv = es_kv.enter_context(
                tc.tile_pool(name=f"kv_{layer}", bufs=1))
            KT = [kv.tile([128, S], dt.bfloat16, tag=f"kt{h}",
                          name=f"kt{h}") for h in range(H)]
            VT = [kv.tile([128, F], dt.bfloat16, tag=f"vt{j}",
                          name=f"vt{j}") for j in range(NT)]
            QT = [kv.tile([128, NLOC], dt.bfloat16, tag=f"qt{h}",
                          name=f"qt{h}") for h in range(H)]
            if layer == 0:
                def sf(blk, i, c0, w):
                    return srcfull[blk, i * 128:(i + 1) * 128, c0:c0 + w]
            else:
                def sf(blk, i, c0, w):
                    return gout[blk, i * 128:(i + 1) * 128, c0:c0 + w]

            bkc_t = [ldbias(f"bkc{h}", bkc[layer, h * 128:(h + 1) * 128, :])
                     for h in range(H)]
            bqc_t = [ldbias(f"bqc{h}", bqc[layer, h * 128:(h + 1) * 128, :])
                     for h in range(H)]

            # ---- phase A1: K^T and V over full S ----
            with tc.tile_pool(name=f"a1_{layer}", bufs=1) as a1, \
                 tc.tile_pool(name=f"a1s_{layer}", bufs=3) as a1s, \
                 tc.tile_pool(name=f"psA_{layer}", bufs=2,
                              space="PSUM") as psA, \
                 tc.tile_pool(name=f"psK_{layer}", bufs=2,
                              space="PSUM") as psK:
                wk8 = []
                for i in range(NF):
                    eng = nc.gpsimd if i % 2 == 0 else nc.sync
                    wt_ = a1.tile([128, F], dt.bfloat16, tag=f"wk{i}",
                                  name=f"wk{i}")
                    eng.dma_start(wt_[:],
                                  wkc[layer, i * 128:(i + 1) * 128, :])
                    wk8.append(wt_)
                # block-0 stream DMAs go out before the wv loads so the
                # first K matmuls aren't stuck behind them in the DMA FIFO
                blk0 = SK_BLOCKS[0]
                srcf0 = []
                for i in range(NF):
                    st_ = a1s.tile([128, 512], dt.bfloat16, tag=f"sf{i}",
                                   name=f"sf{i}")
                    nc.sync.dma_start(st_[:, :blk0[2]],
                                      sf(blk0[0], i, blk0[1], blk0[2]))
                    srcf0.append(st_)
                wv8 = []
                for i in range(NF):
                    vt_ = a1.tile([128, F], dt.bfloat16, tag=f"wv{i}",
                                  name=f"wv{i}")
                    nc.gpsimd.dma_start(vt_[:],
                                        wv[layer, i * 128:(i + 1) * 128, :])
                    wv8.append(vt_)
                for bi, (blk, c0, w, t0) in enumerate(SK_BLOCKS):
                    if bi == 0:
                        srcf = srcf0
                    else:
                        srcf = []
                        for i in range(NF):
                            st_ = a1s.tile([128, 512], dt.bfloat16,
                                           tag=f"sf{i}", name=f"sf{i}")
                            nc.sync.dma_start(st_[:, :w], sf(blk, i, c0, w))
                            srcf.append(st_)
                    for h in range(H):
                        pk = psK.tile([128, 512], dt.float32, tag="km",
                                      name="km")
                        for i in range(NF):
                            nc.tensor.matmul(
                                pk[:, :w], wk8[i][:, h * 128:(h + 1) * 128],
                                srcf[i][:, :w],
                                start=(i == 0), stop=(i == NF - 1))
                        nc.scalar.activation(
                            KT[h][:, t0 * 128:t0 * 128 + w], pk[:, :w],
                            AF.Identity, bias=bkc_t[h][:])
                    off = 0
                    while off < w:
                        wt_ = min(128, w - off)
                        jt = t0 + off // 128
                        for ofb in range(2):
                            pv = psA.tile([128, 512], dt.float32, tag="va",
                                          name="va")
                            for i in range(NF):
                                nc.tensor.matmul(
                                    pv[:wt_, :], srcf[i][:, off:off + wt_],
                                    wv8[i][:, ofb * 512:(ofb + 1) * 512],
                                    start=(i == 0), stop=(i == NF - 1))
                            nc.vector.tensor_scalar_mul(
                                VT[jt][:wt_, ofb * 512:(ofb + 1) * 512],
                                pv[:wt_, :], 1.0)
                        off += wt_

            if layer == 0:
                init_src()

            # ---- phase A2: Q^T over local tokens ----
            with tc.tile_pool(name=f"a2_{layer}", bufs=1) as a2, \
                 tc.tile_pool(name=f"psQ_{layer}", bufs=2,
                              space="PSUM") as psQ:
                wq8 = []
                for i in range(NF):
                    qt_ = a2.tile([128, F], dt.bfloat16, tag=f"wq{i}",
                                  name=f"wq{i}")
                    nc.gpsimd.dma_start(
                        qt_[:], wqc[layer, i * 128:(i + 1) * 128, :])
                    wq8.append(qt_)
                for sq in range(NSQ):
                    cols = slice(sq * SQB, (sq + 1) * SQB)
                    colsb = slice(1 + sq * SQB, 1 + (sq + 1) * SQB)
                    for h in range(H):
                        pq = psQ.tile([128, 512], dt.float32, tag="qm",
                                      name="qm")
                        for i in range(NF):
                            nc.tensor.matmul(
                                pq[:, :SQB], wq8[i][:, h * 128:(h + 1) * 128],
                                SRCB[i][:, colsb],
                                start=(i == 0), stop=(i == NF - 1))
                        nc.scalar.activation(
                            QT[h][:, cols], pq[:, :SQB],
                            AF.Identity, bias=bqc_t[h][:])

            # ---- phase B: attention ----
            bc = es_bc.enter_context(
                tc.tile_pool(name=f"bc_{layer}", bufs=1, side="right"))
            O8 = [bc.tile([128, NLOC], dt.bfloat16, tag=f"o8{h}",
                          name=f"o8{h}") for h in range(H)]
            wo8 = []
            for i in range(NF):
                eng = nc.gpsimd if i % 2 == 0 else nc.sync
                wt_ = bc.tile([128, F], dt.bfloat16, tag=f"wo{i}",
                              name=f"wo{i}")
                eng.dma_start(wt_[:], wo[layer, i * 128:(i + 1) * 128, :])
                wo8.append(wt_)
            l1_8 = []
            for i in range(NF):
                lt = bc.tile([128, DFF], dt.bfloat16, tag=f"l1_{i}",
                             name=f"l1_{i}")
                nc.gpsimd.dma_start(lt[:],
                                    wl1[layer, i * 128:(i + 1) * 128, :])
                l1_8.append(lt)
            l2_1 = bc.tile([128, F], dt.bfloat16, tag="l2", name="l2")
            nc.sync.dma_start(l2_1[:], wl2[layer, :, :])

            with tc.tile_pool(name=f"b_{layer}", bufs=3) as bp, \
                 tc.tile_pool(name=f"b2_{layer}", bufs=2) as bp2, \
                 tc.tile_pool(name=f"psS_{layer}", bufs=3,
                              space="PSUM") as psS, \
                 tc.tile_pool(name=f"psO_{layer}", bufs=2,
                              space="PSUM") as psO, \
                 tc.tile_pool(name=f"psR_{layer}", bufs=2,
                              space="PSUM") as psR:
                STRIPES = [(0, 512), (512, 512), (1024, NLOC - 1024)]
                for h in range(H):
                    for sq0, sw in STRIPES:
                        cols = slice(sq0, sq0 + sw)
                        po = psO.tile([128, 512], dt.float32, tag="O",
                                      name="O")
                        psum = psR.tile([1, 512], dt.float32, tag="R",
                                        name="R")
                        for jt, (blk, c0, wt_, tix) in enumerate(SK_TILES):
                            pS = psS.tile([128, 512], dt.float32, tag="S",
                                          name="S")
                            nc.tensor.matmul(
                                pS[:wt_, :sw],
                                KT[h][:, tix * 128:tix * 128 + wt_],
                                QT[h][:, cols], start=True, stop=True)
                            es = bp.tile([128, 512], dt.bfloat16, tag="es",
                                         name="es")
                            nc.scalar.activation(es[:wt_, :sw], pS[:wt_, :sw],
                                                 AF.Exp, scale=SCALE)
                            nc.tensor.matmul(
                                psum[:, :sw], ones_bf[:wt_, :],
                                es[:wt_, :sw],
                                start=(jt == 0), stop=(jt == NT - 1),
                                skip_group_check=True)
                            nc.tensor.matmul(
                                po[:, :sw],
                                VT[tix][:wt_, h * 128:(h + 1) * 128],
                                es[:wt_, :sw],
                                start=(jt == 0), stop=(jt == NT - 1),
                                skip_group_check=True)
                        sums = bp2.tile([1, 512], dt.float32, tag="sums",
                                        name="sums")
                        nc.vector.tensor_scalar_add(sums[:, :sw],
                                                    psum[:, :sw], 0.0)
                        rec = bp2.tile([1, 512], dt.float32, tag="rec",
                                       name="rec")
                        nc.vector.reciprocal(rec[:, :sw], sums[:, :sw])
                        rbc = bp2.tile([128, 512], dt.float32, tag="rbc",
                                       name="rbc", bufs=1)
                        nc.gpsimd.partition_broadcast(rbc[:, :sw],
                                                      rec[:, :sw])
                        nc.vector.scalar_tensor_tensor(
                            O8[h][:, cols], po[:, :sw], 1.0, rbc[:, :sw],
                            OP.mult, OP.mult)

            es_kv.close()   # KT/VT/QT dead after attention

            # conv weights land in the space KT/VT/QT vacated; the DMAs
            # overlap phase C and the conv blocks interleave with it.
            last = layer == L - 1
            cvw = None
            if last:
                cvw = ctx.enter_context(tc.tile_pool(name="cvw", bufs=1))
                em8 = []
                for k in range(3):
                    row = []
                    for i in range(NF):
                        eng = nc.gpsimd if i % 2 == 0 else nc.sync
                        et = cvw.tile([128, F], dt.bfloat16,
                                      tag=f"em{k}_{i}", name=f"em{k}_{i}")
                        eng.dma_start(
                            et[:], wemb[k, i * 128:(i + 1) * 128, :])
                        row.append(et)
                    em8.append(row)
                bemb_t = [ldbias(f"bemb{i}",
                                 bemb[i * 128:(i + 1) * 128, :])
                          for i in range(NF)]

            # ---- phase C: out-proj + LN1 + FFN + LN2 (+ conv blocks) ----
            with tc.tile_pool(name=f"c_{layer}", bufs=1) as cp, \
                 tc.tile_pool(name=f"c2_{layer}", bufs=2) as cp2, \
                 tc.tile_pool(name=f"psC_{layer}", bufs=3,
                              space="PSUM") as psC, \
                 tc.tile_pool(name=f"psRC_{layer}", bufs=2,
                              space="PSUM") as psRC:
                bo_t = [ldbias(f"bo{i}", bo[layer, i * 128:(i + 1) * 128, :])
                        for i in range(NF)]
                bl1_t = ldbias("bl1", bl1[layer, :, :])
                bl2_t = [ldbias(f"bl2{i}", bl2[layer, i * 128:(i + 1) * 128, :])
                         for i in range(NF)]
                g1_t = [ldbias(f"g1{i}", g1[layer, i * 128:(i + 1) * 128, :])
                        for i in range(NF)]
                be1_t = [ldbias(f"be1{i}", be1[layer, i * 128:(i + 1) * 128, :])
                         for i in range(NF)]
                g2_t = [ldbias(f"g2{i}", g2[layer, i * 128:(i + 1) * 128, :])
                        for i in range(NF)]
                be2_t = [ldbias(f"be2{i}", be2[layer, i * 128:(i + 1) * 128, :])
                         for i in range(NF)]

                def layernorm(sq, y8, gt, bt, masked):
                    """y8: 8 f32r tiles (128,SQB). Writes SRC + SRCB."""
                    cols = slice(sq * SQB, (sq + 1) * SQB)
                    colsb = slice(1 + sq * SQB, 1 + (sq + 1) * SQB)
                    pst = psRC.tile([1, SQB], dt.float32, tag="cr",
                                    name="cr")
                    psq = psRC.tile([1, SQB], dt.float32, tag="cr",
                                    name="cr")
                    for i in range(NF):
                        y2 = cp2.tile([128, SQB], dt.float32r, tag="y2",
                                      name="y2", bufs=1)
                        nc.scalar.activation(y2[:], y8[i][:], AF.Square)
                        nc.tensor.matmul(pst[:], ones_r[:], y8[i][:],
                                         start=(i == 0), stop=(i == NF - 1),
                                         skip_group_check=True)
                        nc.tensor.matmul(psq[:], ones_r[:], y2[:],
                                         start=(i == 0), stop=(i == NF - 1),
                                         skip_group_check=True)
                    mu = cp2.tile([1, SQB], dt.float32, tag="mu", name="mu")
                    nc.vector.tensor_scalar_mul(mu[:], pst[:], 1.0 / F)
                    mu2 = cp2.tile([1, SQB], dt.float32, tag="mu2",
                                   name="mu2")
                    nc.vector.scalar_tensor_tensor(mu2[:], mu[:], 1.0, mu[:],
                                                   OP.mult, OP.mult)
                    var = cp2.tile([1, SQB], dt.float32, tag="var",
                                   name="var")
                    nc.vector.scalar_tensor_tensor(var[:], psq[:], 1.0 / F,
                                                   mu2[:], OP.mult,
                                                   OP.subtract)
                    std = cp2.tile([1, SQB], dt.float32, tag="std",
                                   name="std")
                    nc.scalar.activation(std[:], var[:], AF.Sqrt,
                                         bias=eps_t[:])
                    rstd = cp2.tile([1, SQB], dt.float32, tag="rstd",
                                    name="rstd")
                    nc.vector.reciprocal(rstd[:], std[:])
                    mu_bc = cp2.tile([128, SQB], dt.float32, tag="mubc",
                                     name="mubc", bufs=1)
                    nc.gpsimd.partition_broadcast(mu_bc[:], mu[:])
                    rs_bc = cp2.tile([128, SQB], dt.float32, tag="rsbc",
                                     name="rsbc", bufs=1)
                    nc.gpsimd.partition_broadcast(rs_bc[:], rstd[:])
                    for i in range(NF):
                        t1 = cp2.tile([128, SQB], dt.float32, tag="t1",
                                      name="t1")
                        nc.vector.scalar_tensor_tensor(
                            t1[:], y8[i][:], 0.0, mu_bc[:],
                            OP.add, OP.subtract)
                        t2 = cp2.tile([128, SQB], dt.float32, tag="t2",
                                      name="t2")
                        nc.vector.scalar_tensor_tensor(
                            t2[:], t1[:], gt[i][:], rs_bc[:],
                            OP.mult, OP.mult)
                        nc.scalar.activation(SRC[i][:, cols], t2[:],
                                             AF.Identity, bias=bt[i][:])
                        if masked:
                            # final layer: conv input = masked src
                            nc.vector.scalar_tensor_tensor(
                                SRCB[i][:, colsb], SRC[i][:, cols], 1.0,
                                mbc[:, cols], OP.mult, OP.mult)
                        else:
                            nc.scalar.copy(SRCB[i][:, colsb],
                                           SRC[i][:, cols])

                def colsb2(sq):
                    return slice(sq * SQB, (sq + 1) * SQB)

                def conv_block(tb):
                    base = tb * SQB
                    for of in range(NF):
                        pc = psC.tile([128, 512], dt.float32, tag="cm",
                                      name="cm")
                        first = True
                        for k in range(3):
                            for i in range(NF):
                                nc.tensor.matmul(
                                    pc[:, :SQB], em8[k][i][:, of * 128:(of + 1) * 128],
                                    SRCB[i][:, base + k:base + k + SQB],
                                    start=first,
                                    stop=(k == 2 and i == NF - 1),
                                    skip_group_check=True)
                                first = False
                        ob = cp2.tile([128, SQB], dt.float32, tag="ob",
                                      name="ob")
                        nc.scalar.activation(ob[:], pc[:, :SQB], AF.Relu,
                                             bias=bemb_t[of][:])
                        nc.sync.dma_start(
                            out[of * 128:(of + 1) * 128, base:base + SQB],
                            ob[:])

                def outproj(sq):
                    cols = slice(sq * SQB, (sq + 1) * SQB)
                    y8 = [cp.tile([128, SQB], dt.float32r, tag=f"y{i}",
                                  name=f"y{i}", bufs=2) for i in range(NF)]
                    for i in range(NF):
                        pa = psC.tile([128, 512], dt.float32, tag="cm",
                                      name="cm")
                        for h in range(H):
                            nc.tensor.matmul(
                                pa[:, :SQB], wo8[h][:, i * 128:(i + 1) * 128],
                                O8[h][:, cols],
                                start=(h == 0), stop=(h == H - 1))
                        nc.vector.scalar_tensor_tensor(
                            y8[i][:], pa[:, :SQB], bo_t[i][:],
                            SRC[i][:, cols], OP.add, OP.add)
                    return y8

                for sq in range(NSQ):
                    cols = slice(sq * SQB, (sq + 1) * SQB)
                    colsb = slice(1 + sq * SQB, 1 + (sq + 1) * SQB)
                    y8 = outproj(sq)
                    layernorm(sq, y8, g1_t, be1_t, masked=False)

                    ph = psC.tile([128, 512], dt.float32, tag="cm",
                                  name="mm")
                    for i in range(NF):
                        nc.tensor.matmul(ph[:, :SQB], l1_8[i][:],
                                         SRCB[i][:, colsb],
                                         start=(i == 0), stop=(i == NF - 1))
                    h1 = cp2.tile([128, SQB], dt.bfloat16, tag="h1",
                                  name="h1")
                    nc.scalar.activation(h1[:], ph[:, :SQB], AF.Relu,
                                         bias=bl1_t[:])
                    for i in range(NF):
                        pf = psC.tile([128, 512], dt.float32, tag="cm",
                                      name="mm")
                        nc.tensor.matmul(pf[:, :SQB],
                                         l2_1[:, i * 128:(i + 1) * 128],
                                         h1[:], start=True, stop=True)
                        nc.vector.scalar_tensor_tensor(
                            y8[i][:], pf[:, :SQB], bl2_t[i][:],
                            SRC[i][:, cols], OP.add, OP.add)
                    layernorm(sq, y8, g2_t, be2_t, masked=last)
                    if layer == 0:
                        for i in range(NF):
                            nc.sync.dma_start(
                                gin[i * 128:(i + 1) * 128, colsb2(sq)],
                                SRCB[i][:, 1 + sq * SQB:1 + (sq + 1) * SQB])
                    if last and sq >= 1:
                        conv_block(sq - 1)
                if last:
                    conv_block(NSQ - 1)

            es_bc.close()   # O8 dead

            # ---- gather between layers ----
            if layer == 0:
                if no_cc:
                    nc.sync.dma_start(gout[0], gin[:])
                    nc.sync.dma_start(gout[1], gin[:])
                else:
                    nc.gpsimd.collective_compute(
                        "AllGather", OP.bypass,
                        replica_groups=REPLICA_GROUPS,
                        ins=[gin[:]], outs=[gout[:]])


def _prep_host(inputs):
    """Fold weights (fp64), build per-core input maps."""
    import ml_dtypes
    bf16 = ml_dtypes.bfloat16
    f32 = np.float32
    g = {k: np.asarray(v) for k, v in inputs.items()}
    x, protos = g["x"], g["prototypes"]
    in_w, in_b = g["in_w"], g["in_b"]

    wkc = np.stack([(in_w[l, F:2 * F].astype(np.float64)
                     @ g["wk"][l].astype(np.float64)).T for l in range(L)])
    bkc = np.stack([(in_w[l, F:2 * F].astype(np.float64)
                     @ g["bk"][l].astype(np.float64)
                     + in_b[l, F:2 * F]) for l in range(L)])
    wqc = np.stack([(in_w[l, :F].astype(np.float64)
                     @ g["wq"][l].astype(np.float64)).T for l in range(L)])
    bqc = np.stack([(in_w[l, :F].astype(np.float64)
                     @ g["bq"][l].astype(np.float64)
                     + in_b[l, :F]) for l in range(L)])

    shared = {
        "wkc": wkc.astype(bf16),
        "wqc": wqc.astype(bf16),
        "wv": np.ascontiguousarray(
            in_w[:, 2 * F:].transpose(0, 2, 1)).astype(bf16),
        "wo": np.ascontiguousarray(
            g["out_w"].transpose(0, 2, 1)).astype(bf16),
        "wl1": np.ascontiguousarray(
            g["l1_w"].transpose(0, 2, 1)).astype(bf16),
        "wl2": np.ascontiguousarray(
            g["l2_w"].transpose(0, 2, 1)).astype(bf16),
        "wemb": np.ascontiguousarray(
            g["emb_w"].transpose(2, 1, 0)).astype(bf16),
        "bkc": bkc.astype(f32).reshape(L, F, 1),
        "bqc": bqc.astype(f32).reshape(L, F, 1),
        "bo": np.stack([
            (g["out_w"][l].astype(np.float64)
             @ in_b[l, 2 * F:].astype(np.float64) + g["out_b"][l])
            for l in range(L)]).astype(f32).reshape(L, F, 1),
        "bl1": g["l1_b"].reshape(L, DFF, 1).astype(f32),
        "bl2": g["l2_b"].reshape(L, F, 1).astype(f32),
        "g1": g["ln1_g"].reshape(L, F, 1).astype(f32),
        "be1": g["ln1_b"].reshape(L, F, 1).astype(f32),
        "g2": g["ln2_g"].reshape(L, F, 1).astype(f32),
        "be2": g["ln2_b"].reshape(L, F, 1).astype(f32),
        "bemb": g["emb_b"].reshape(F, 1).astype(f32),
    }

    proto_mat = protos.reshape(F, C)   # raw reshape (matches reference)
    mask0 = np.ones((1, NLOC), bf16)
    mask1 = np.zeros((1, NLOC), bf16)
    mask1[0, :T - R1] = 1.0

    in_maps = []
    for c in range(8):
        b, half = c // 2, c % 2
        src0 = np.concatenate([x[b].T, proto_mat], axis=1)   # (F, S)
        blocks = np.stack([src0[:, :NLOC], src0[:, R1:]])    # (2, F, NLOC)
        m = dict(shared)
        m["srcfull"] = blocks.astype(bf16)
        m["srcloc"] = np.ascontiguousarray(blocks[half]).astype(f32)
        m["maskin"] = mask0 if half == 0 else mask1
        in_maps.append(m)
    return in_maps


def run(inputs, no_cc=False, **kw):
    nc = _build_program(no_cc=no_cc)
    in_maps = _prep_host(inputs)
    res = run_bass_kernel_spmd(nc, in_maps, core_ids=list(range(8)), **kw)
    y = np.zeros((B, F, T), np.float32)
    for b in range(B):
        o0 = res.results[2 * b]["out"]
        o1 = res.results[2 * b + 1]["out"]
        y[b, :, :CONV_SPLIT] = o0[:, :CONV_SPLIT]
        y[b, :, CONV_SPLIT:] = o1[:, CONV_SPLIT - R1:T - R1]
    return y, res


def kernel(**inputs) -> np.ndarray:
    y, _ = run(inputs)
    return y



# revision 39
# speedup vs baseline: 1.5503x; 1.5503x over previous
"""Trainium2 Bass kernel for nn_DecoupledCls (RAB transformer + conv head).

Sharding: 8 cores = (batch b, sequence half). Core (b, 0) owns tokens
[0, 1152); core (b, 1) owns tokens [1096, 2248) of the S = 2248 token
sequence (T=2048 video tokens + C=200 prototype tokens). All per-token
ops run on local tokens; K/V are computed redundantly over the full S
streamed from DRAM (layer 0: host upload; layer 1: fp8 pairwise
AllGather between the two half-cores of a batch).

vs baseline: fp8e4 DoubleRow matmuls (2 k-subtiles of 128 per
instruction, 0.5 cyc/row) for the K/V/Q projections and FFN-l1
(contraction F=1024) and for the attention PV + softmax-denominator
(ones) matmuls (contraction over key-tile pairs). QK runs fp8 non-DR
(contraction dh=128). Keys are zero-padded to 2304 so all 18 key tiles
are uniform 128 wide (a masked-ones operand and zeroed V rows exclude
the 56 pad keys). exp activations merge per key-tile pair ([128,2,384]
psum -> fp8), depth-2 software pipelined against the ones/PV matmuls.
The residual stream is bf16 (SRCB) + fp8 pairs (S8); the fp32 residual
copy is dropped. Engine placement keeps the Activation engine exp-only
during attention (K/Q drains run there in phase A, LN rstd uses a DVE
bit-trick rsqrt, LN writes run on DVE, squares on Pool). Phase B
(ACT-bound attention) software-pipelines with phase C (PE-bound
out-proj/FFN/LN) by interleaving C slices of stripe sq-1 between the
heads of stripe sq; each layer's final C stripe crosses the layer
boundary and interleaves with the next layer's Q-projection (or with
the final conv), and the next layer's weights prefetch during the
previous layer's attention.

Host-side folding: the query/key 1x1 convs are fused into the MHA
in-projections (wQC = wQ_mha @ wq_conv etc., folded in fp64); fp8
weights are pre-scaled by 32 (descaled in the psum drain) to stay in
e4m3 normal range.
"""
import math
import numpy as np

import concourse.bacc as bacc
import concourse.mybir as mybir
import concourse.tile as tile
from concourse.bass_utils import run_bass_kernel_spmd

dt = mybir.dt
AF = mybir.ActivationFunctionType
OP = mybir.AluOpType
DR = mybir.MatmulPerfMode.DoubleRow

L, B, T, F, C, H, DFF = 2, 4, 2048, 1024, 200, 8, 128
S = T + C                  # 2248
SP = 2304                  # padded key count (18*128); keys [2248:2304) dead
NLOC = 1152                # local tokens per core (9*128)
R1 = S - NLOC              # 1096: start of half-1 local range
CONV_SPLIT = 1120          # conv output ownership split
SQB = 384                  # query-stripe width (3 stripes cover NLOC)
NSQ = NLOC // SQB
NF = F // 128              # 8 feature tiles
NP = NF // 2               # 4 fp8 feature-tile pairs
NT = 18                    # key tiles (128 wide, last 56 cols dead)
NJ2 = NT // 2              # 9 key-tile pairs
SC = 32.0                  # fp8 weight pre-scale (host); descaled on drain
ISC = 1.0 / SC
SCALE = float(1.0 / math.sqrt(128.0))
REPLICA_GROUPS = [[0, 1], [2, 3], [4, 5], [6, 7]]

# A1 chunks: (block, col0, width, tile0). Enumerates all 2248 keys once:
# block 0 = tokens [0,1152), block 1 cols [56,1152) = tokens [1152,2248).
# Phase a covers source cols [0,768) of both blocks (first AllGather
# chunk), phase b the rest, so layer-1 K/V can start before the second
# gather chunk lands. All chunks are key-tile aligned.
SKA = [
    (0, 0, 512, 0), (0, 512, 256, 4), (1, 56, 512, 9), (1, 568, 128, 13),
]
SKB = [
    (0, 768, 384, 6), (1, 696, 128, 14), (1, 824, 328, 15),
]
SK_BLOCKS = SKA + SKB

_CACHE = {}


def _build_program(no_cc=False):
    key = ("nc", no_cc)
    if key in _CACHE:
        return _CACHE[key]

    nc = bacc.Bacc("TRN2", target_bir_lowering=False, debug=False,
                   num_devices=8)

    def din(name, shape, dty=dt.float32):
        return nc.dram_tensor(name, list(shape), dty, kind="ExternalInput")

    srcf8 = din("srcf8", (2, NP, 128, 2, NLOC), dt.float8e4)
    srcloc8 = din("srcloc8", (NP, 128, 2, NLOC), dt.float8e4)
    srclocb = din("srclocb", (F, NLOC), dt.bfloat16)
    maskin = din("maskin", (1, NLOC), dt.bfloat16)
    wkc = din("wkc", (L, NP, 128, 2 * F), dt.float8e4)
    wv = din("wv", (L, NP, 128, 2 * F), dt.float8e4)
    wqc = din("wqc", (L, NP, 128, 2 * F), dt.float8e4)
    wl1 = din("wl1", (L, NP, 128, 2 * DFF), dt.float8e4)
    wo = din("wo", (L, F, F), dt.bfloat16)
    wl2 = din("wl2", (L, DFF, F), dt.bfloat16)
    wemb = din("wemb", (3, F, F), dt.bfloat16)
    bkc = din("bkc", (L, F, 1))
    bqc = din("bqc", (L, F, 1))
    bo = din("bo", (L, F, 1))
    bl1 = din("bl1", (L, DFF, 1))
    bl2 = din("bl2", (L, F, 1))
    g1 = din("g1", (L, F, 1))
    be1 = din("be1", (L, F, 1))
    g2 = din("g2", (L, F, 1))
    be2 = din("be2", (L, F, 1))
    bemb = din("bemb", (F, 1))
    out = nc.dram_tensor("out", [F, NLOC], dt.float32, kind="ExternalOutput")

    with tile.TileContext(nc) as tc:
        _emit(nc, tc, locals(), no_cc=no_cc)
    nc.compile()
    _CACHE[key] = nc
    return nc


def _emit(nc, tc, t, no_cc=False):
    from contextlib import ExitStack
    srcf8, srcloc8, srclocb, maskin = (t["srcf8"], t["srcloc8"],
                                       t["srclocb"], t["maskin"])
    wkc, wv, wqc, wo, wl1, wl2, wemb = (t["wkc"], t["wv"], t["wqc"], t["wo"],
                                        t["wl1"], t["wl2"], t["wemb"])
    bkc, bqc, bo, bl1, bl2 = (t["bkc"], t["bqc"], t["bo"], t["bl1"],
                              t["bl2"])
    g1, be1, g2, be2, bemb, out = (t["g1"], t["be1"], t["g2"], t["be2"],
                                   t["bemb"], t["out"])

    ctx = ExitStack()
    with ctx:
        # ---------------- hoisted pools ----------------
        cn = ctx.enter_context(tc.tile_pool(name="cn", bufs=1))
        bias = ctx.enter_context(tc.tile_pool(name="bias", bufs=1))
        res = ctx.enter_context(tc.tile_pool(name="res", bufs=1))
        dram = ctx.enter_context(tc.tile_pool(name="dram", bufs=1,
                                              space="DRAM"))
        aw = ctx.enter_context(tc.tile_pool(name="aw", bufs=1))
        bcp = ctx.enter_context(tc.tile_pool(name="bcp", bufs=1,
                                             side="right"))
        cp = ctx.enter_context(tc.tile_pool(name="cp", bufs=1))
        cp2 = ctx.enter_context(tc.tile_pool(name="cp2", bufs=2))
        psC = ctx.enter_context(tc.tile_pool(name="psC", bufs=2,
                                             space="PSUM"))

        # ---------------- resident activations (DMAs first) ----------------
        SRCB = [res.tile([128, NLOC + 2], dt.bfloat16, tag=f"srcb{i}",
                         name=f"srcb{i}") for i in range(NF)]
        S8 = [res.tile([128, 2, NLOC], dt.float8e4, tag=f"s8_{j}",
                       name=f"s8_{j}") for j in range(NP)]
        for j in range(NP):
            nc.sync.dma_start(S8[j][:], srcloc8[j])

        # A-phase weights, reloaded per layer into the same buffers
        wq4 = [aw.tile([128, 2, F], dt.float8e4, tag=f"wq{j}",
                       name=f"wq{j}") for j in range(NP)]
        wk4 = [aw.tile([128, 2, F], dt.float8e4, tag=f"wk{j}",
                       name=f"wk{j}") for j in range(NP)]
        wv4 = [aw.tile([128, 2, F], dt.float8e4, tag=f"wv{j}",
                       name=f"wv{j}") for j in range(NP)]

        def load_awqk(layer):
            for j in range(NP):
                nc.sync.dma_start(wq4[j][:], wqc[layer, j])
            for j in range(NP):
                nc.sync.dma_start(wk4[j][:], wkc[layer, j])

        def load_awv(layer):
            for j in range(NP):
                nc.sync.dma_start(wv4[j][:], wv[layer, j])

        load_awqk(0)
        load_awv(0)

        # B/C-phase weights + attention output, reused per layer
        O8 = [bcp.tile([128, NLOC], dt.bfloat16, tag=f"o8{h}",
                       name=f"o8{h}") for h in range(H)]
        wo8 = [bcp.tile([128, F], dt.bfloat16, tag=f"wo{i}",
                        name=f"wo{i}") for i in range(NF)]
        l1_4 = [bcp.tile([128, 2, DFF], dt.float8e4, tag=f"l1_{j}",
                         name=f"l1_{j}") for j in range(NP)]
        l2_1 = bcp.tile([128, F], dt.bfloat16, tag="l2", name="l2")

        def load_bc(layer):
            for i in range(NF):
                nc.sync.dma_start(wo8[i][:],
                                  wo[layer, i * 128:(i + 1) * 128, :])
            for j in range(NP):
                nc.sync.dma_start(l1_4[j][:], wl1[layer, j])
            nc.sync.dma_start(l2_1[:], wl2[layer, :, :])

        # conv-input mask (zeroes prototype tokens on the half-1 cores)
        mrow = res.tile([1, NLOC], dt.bfloat16, tag="mrow", name="mrow")
        nc.sync.dma_start(mrow[:], maskin[:])

        # ---------------- constants / pads ----------------
        # DoubleRow lhsT outer stride must be even and 16B-aligned, so the
        # ones vectors are padded to 16 columns and sliced [:, :, 0:1].
        ones8 = cn.tile([128, 2, 16], dt.float8e4)
        nc.gpsimd.memset(ones8[:], 1.0)
        ones8m = cn.tile([128, 2, 16], dt.float8e4)
        nc.gpsimd.memset(ones8m[:], 1.0)
        # zero dead-key rows [72:128) of subtile 1 (partition starts must be
        # 32-aligned, so zero [64:128) then restore [64:72))
        nc.gpsimd.memset(ones8m[64:128, 1, :], 0.0)
        nc.gpsimd.memset(ones8m[64:72, 1, :], 1.0)
        ones_f = cn.tile([128, 1], dt.float32)
        nc.gpsimd.memset(ones_f[:], 1.0)
        ones_r = cn.tile([128, 1], dt.float32r)
        nc.scalar.copy(ones_r[:], ones_f[:])
        eps_t = cn.tile([1, 1], dt.float32)
        nc.gpsimd.memset(eps_t[:], 1e-5)

        for i in range(NF):
            nc.vector.memset(SRCB[i][:, 0:1], 0.0)
            nc.vector.memset(SRCB[i][:, NLOC + 1:NLOC + 2], 0.0)
        mbc = res.tile([128, NLOC], dt.bfloat16, tag="mbc", name="mbc")
        nc.gpsimd.partition_broadcast(mbc[:], mrow[:])

        GCA = 768   # first gather chunk covers local cols [0, 768)
        gin_a = dram.tile([NP, 128, 2, GCA], dt.float8e4)
        gout_a = dram.tile([2, NP, 128, 2, GCA], dt.float8e4)
        gin_b = dram.tile([NP, 128, 2, NLOC - GCA], dt.float8e4)
        gout_b = dram.tile([2, NP, 128, 2, NLOC - GCA], dt.float8e4)

        def gather(gi, go):
            if no_cc:
                nc.sync.dma_start(go[0], gi[:])
                nc.sync.dma_start(go[1], gi[:])
            else:
                nc.gpsimd.collective_compute(
                    "AllGather", OP.bypass,
                    replica_groups=REPLICA_GROUPS,
                    ins=[gi[:]], outs=[go[:]])

        def gsrc(layer, blk, j, c0, w):
            """K/V moving-source AP for chunk cols [c0, c0+w)."""
            if layer == 0:
                return srcf8[blk, j, :, :, c0:c0 + w]
            if c0 + w <= GCA:
                return gout_a[blk, j, :, :, c0:c0 + w]
            assert c0 >= GCA
            return gout_b[blk, j, :, :, c0 - GCA:c0 - GCA + w]

        def ldbias(pool_tag, src_ap):
            b = bias.tile([src_ap.shape[0], 1], dt.float32, tag=pool_tag,
                          name=pool_tag)
            nc.sync.dma_start(b[:], src_ap)
            return b

        def interleave(primary, fillers):
            """Emit primary[i]() with fillers popped round-robin after
            each; any leftover fillers run at the end."""
            fi = 0
            for p in primary:
                p()
                if fi < len(fillers):
                    fillers[fi]()
                    fi += 1
            while fi < len(fillers):
                fillers[fi]()
                fi += 1

        pend_c = []   # C-tail slices carried across the layer boundary

        for layer in range(L):
            last = layer == L - 1
            es_kv = ExitStack()
            kv = es_kv.enter_context(
                tc.tile_pool(name=f"kv_{layer}", bufs=1))
            KT = [kv.tile([128, SP], dt.float8e4, tag=f"kt{h}",
                          name=f"kt{h}") for h in range(H)]
            VT = [kv.tile([128, 2, F], dt.float8e4, tag=f"vt{j}",
                          name=f"vt{j}") for j in range(NJ2)]
            QT = [kv.tile([128, NLOC], dt.float8e4, tag=f"qt{h}",
                          name=f"qt{h}") for h in range(H)]
            # dead-key padding: zero K columns and V rows for [2248:2304)
            for h in range(H):
                nc.vector.memset(KT[h][:, S:SP], 0.0)
            # dead V rows: zero [64:128) of the last subtile; rows [64:72)
            # are rewritten by the V drain afterwards
            nc.vector.memset(VT[NJ2 - 1][64:128, 1, :], 0.0)

            bkc_t = [ldbias(f"bkc{h}", bkc[layer, h * 128:(h + 1) * 128, :])
                     for h in range(H)]
            bqc_t = [ldbias(f"bqc{h}", bqc[layer, h * 128:(h + 1) * 128, :])
                     for h in range(H)]

            # ---- phase A (Q-proj + K/V over padded S), interleaved with
            # the previous layer's carried C tail ----
            with tc.tile_pool(name=f"a1s_{layer}", bufs=2) as a1s, \
                 tc.tile_pool(name=f"psQ_{layer}", bufs=2,
                              space="PSUM") as psQ, \
                 tc.tile_pool(name=f"psA_{layer}", bufs=2,
                              space="PSUM") as psA, \
                 tc.tile_pool(name=f"psK_{layer}", bufs=2,
                              space="PSUM") as psK:
                def a2_chunk(sq, hr):
                    def f():
                        cols = slice(sq * SQB, (sq + 1) * SQB)
                        for h in hr:
                            pq = psQ.tile([128, 512], dt.float32, tag="qm",
                                          name="qm")
                            for j in range(NP):
                                nc.tensor.matmul(
                                    pq[:, :SQB],
                                    wq4[j][:, :, h * 128:(h + 1) * 128],
                                    S8[j][:, :, cols],
                                    start=(j == 0), stop=(j == NP - 1),
                                    perf_mode=DR)
                            if h % 2 == 0:
                                nc.scalar.activation(
                                    QT[h][:, cols], pq[:, :SQB],
                                    AF.Identity, bias=bqc_t[h][:],
                                    scale=ISC)
                            else:
                                nc.vector.tensor_scalar(
                                    QT[h][:, cols], pq[:, :SQB], ISC,
                                    bqc_t[h][:], OP.mult, OP.add)
                    return f

                def a1_chunk(blk, c0, w, t0):
                    def f():
                        srcp = []
                        for j in range(NP):
                            st_ = a1s.tile([128, 2, 512], dt.float8e4,
                                           tag=f"sf{j}", name=f"sf{j}")
                            if layer > 0 and c0 < GCA < c0 + w:
                                # chunk straddles the gather split
                                wa = GCA - c0
                                nc.sync.dma_start(
                                    st_[:, :, :wa],
                                    gsrc(layer, blk, j, c0, wa))
                                nc.sync.dma_start(
                                    st_[:, :, wa:w],
                                    gsrc(layer, blk, j, GCA, w - wa))
                            else:
                                nc.sync.dma_start(
                                    st_[:, :, :w],
                                    gsrc(layer, blk, j, c0, w))
                            srcp.append(st_)
                        for h in range(H):
                            pk = psK.tile([128, 512], dt.float32, tag="km",
                                          name="km")
                            for j in range(NP):
                                nc.tensor.matmul(
                                    pk[:, :w],
                                    wk4[j][:, :, h * 128:(h + 1) * 128],
                                    srcp[j][:, :, :w],
                                    start=(j == 0), stop=(j == NP - 1),
                                    perf_mode=DR)
                            kslc = KT[h][:, t0 * 128:t0 * 128 + w]
                            if h % 2 == 0:
                                nc.scalar.activation(
                                    kslc, pk[:, :w], AF.Identity,
                                    bias=bkc_t[h][:], scale=ISC)
                            else:
                                nc.vector.tensor_scalar(
                                    kslc, pk[:, :w], ISC, bkc_t[h][:],
                                    OP.mult, OP.add)
                        off = 0
                        while off < w:
                            wt_ = min(128, w - off)
                            jt = t0 + off // 128
                            for ofb in range(2):
                                pv = psA.tile([128, 512], dt.float32,
                                              tag="va", name="va")
                                for j in range(NP):
                                    nc.tensor.matmul(
                                        pv[:wt_, :],
                                        srcp[j][:, :, off:off + wt_],
                                        wv4[j][:, :,
                                               ofb * 512:(ofb + 1) * 512],
                                        start=(j == 0), stop=(j == NP - 1),
                                        perf_mode=DR)
                                vslc = VT[jt // 2][:wt_, jt % 2,
                                                   ofb * 512:(ofb + 1) * 512]
                                if layer > 0:
                                    nc.scalar.activation(vslc, pv[:wt_, :],
                                                         AF.Identity,
                                                         scale=ISC)
                                else:
                                    nc.vector.tensor_scalar_mul(
                                        vslc, pv[:wt_, :], ISC)
                            off += wt_
                    return f

                a2g = [a2_chunk(sq, hr) for sq in range(NSQ)
                       for hr in (range(0, 4), range(4, H))]
                a1a = [a1_chunk(*c) for c in SKA]
                a1b = [a1_chunk(*c) for c in SKB]

                if pend_c:
                    # A2(sq<2) and phase-a K/V (gathered early) fill the
                    # PE while the carried C tail's DVE chains drain;
                    # A2(2) needs the tail's LN2, phase-b the 2nd gather.
                    s = pend_c
                    seq = (s[0:1] + a2g[0:1] + s[1:2] + a2g[1:2]
                           + s[2:3] + a2g[2:3] + s[3:4] + a2g[3:4]
                           + s[4:5] + a1a[0:1] + s[5:6] + a1a[1:2]
                           + s[6:7] + a1a[2:3] + s[7:8] + a1a[3:4])
                    for f in seq:
                        f()
                    pend_c = []
                    gather(gin_b, gout_b)
                    for f in a2g[4:6] + a1b:
                        f()
                    load_bc(1)
                else:
                    for f in a2g + a1a + a1b:
                        f()

            if layer == 0:
                for i in range(NF):
                    nc.sync.dma_start(SRCB[i][:, 1:NLOC + 1],
                                      srclocb[i * 128:(i + 1) * 128, :])
                load_bc(0)

            # ---- phases B+C, software-pipelined per 384-query stripe ----
            with tc.tile_pool(name=f"b_{layer}", bufs=3) as bp, \
                 tc.tile_pool(name=f"b2_{layer}", bufs=2) as bp2, \
                 tc.tile_pool(name=f"psS_{layer}", bufs=2,
                              space="PSUM") as psS, \
                 tc.tile_pool(name=f"psO_{layer}", bufs=1,
                              space="PSUM") as psO, \
                 tc.tile_pool(name=f"psR_{layer}", bufs=1,
                              space="PSUM") as psR:

                bo_t = [ldbias(f"bo{i}", bo[layer, i * 128:(i + 1) * 128, :])
                        for i in range(NF)]
                bl1_t = ldbias("bl1", bl1[layer, :, :])
                bl2_t = [ldbias(f"bl2{i}", bl2[layer, i * 128:(i + 1) * 128, :])
                         for i in range(NF)]
                g1_t = [ldbias(f"g1{i}", g1[layer, i * 128:(i + 1) * 128, :])
                        for i in range(NF)]
                be1_t = [ldbias(f"be1{i}", be1[layer, i * 128:(i + 1) * 128, :])
                         for i in range(NF)]
                g2_t = [ldbias(f"g2{i}", g2[layer, i * 128:(i + 1) * 128, :])
                        for i in range(NF)]
                be2_t = [ldbias(f"be2{i}", be2[layer, i * 128:(i + 1) * 128, :])
                         for i in range(NF)]

                def _flush_pair(e8, j2, po, psum, h):
                    ot = ones8m if j2 == NJ2 - 1 else ones8
                    nc.tensor.matmul(
                        psum[:, :SQB], ot[:, :, 0:1], e8[:, :, :],
                        start=(j2 == 0), stop=(j2 == NJ2 - 1),
                        perf_mode=DR, skip_group_check=True)
                    nc.tensor.matmul(
                        po[:, :SQB],
                        VT[j2][:, :, h * 128:(h + 1) * 128],
                        e8[:, :, :],
                        start=(j2 == 0), stop=(j2 == NJ2 - 1),
                        perf_mode=DR, skip_group_check=True)

                def b_head(sq, h):
                    """Attention for one (stripe, head): QK, merged exp,
                    ones+PV (DoubleRow over key-tile pairs), O8 write."""
                    cols = slice(sq * SQB, (sq + 1) * SQB)
                    po = psO.tile([128, 512], dt.float32, tag="O", name="O")
                    psum = psR.tile([1, 512], dt.float32, tag="R", name="R")
                    pend = []
                    for j2 in range(NJ2):
                        pS = psS.tile([128, 2, 512], dt.float32, tag="s",
                                      name="s")
                        for sub in range(2):
                            tix = 2 * j2 + sub
                            nc.tensor.matmul(
                                pS[:, sub, :SQB],
                                KT[h][:, tix * 128:(tix + 1) * 128],
                                QT[h][:, cols], start=True, stop=True)
                        e8 = bp.tile([128, 2, SQB], dt.float8e4, tag="e8",
                                     name="e8")
                        nc.scalar.activation(e8[:], pS[:, :, :SQB],
                                             AF.Exp, scale=SCALE)
                        pend.append((e8, j2))
                        # depth-2 pipeline: PE stays 2 QK-pairs ahead of the
                        # exp-consuming ones/PV matmuls, hiding ACT latency
                        if len(pend) == 3:
                            _flush_pair(*pend.pop(0), po, psum, h)
                    while pend:
                        _flush_pair(*pend.pop(0), po, psum, h)
                    sums = bp2.tile([1, SQB], dt.float32, tag="sums",
                                    name="sums", bufs=1)
                    nc.vector.tensor_scalar_add(sums[:], psum[:, :SQB], 0.0)
                    rec = bp2.tile([1, SQB], dt.float32, tag="rec",
                                   name="rec", bufs=1)
                    nc.vector.reciprocal(rec[:], sums[:])
                    rbc = bp2.tile([128, SQB], dt.float32, tag="rbc",
                                   name="rbc", bufs=1)
                    nc.gpsimd.partition_broadcast(rbc[:], rec[:])
                    nc.vector.scalar_tensor_tensor(
                        O8[h][:, cols], po[:, :SQB], 1.0, rbc[:],
                        OP.mult, OP.mult)

                def ln_stats(y8):
                    """16 fp32r ones-matmuls into row 0 of two psC tiles."""
                    psa = psC.tile([128, 512], dt.float32, tag="cm",
                                   name="cm")
                    psb = psC.tile([128, 512], dt.float32, tag="cm",
                                   name="cm")
                    for i in range(NF):
                        y2 = cp2.tile([128, SQB], dt.float32r, tag="y2",
                                      name="y2", bufs=1)
                        nc.gpsimd.tensor_mul(y2[:], y8[i][:], y8[i][:])
                        nc.tensor.matmul(psa[0:1, :SQB], ones_r[:], y8[i][:],
                                         start=(i == 0), stop=(i == NF - 1),
                                         skip_group_check=True)
                        nc.tensor.matmul(psb[0:1, :SQB], ones_r[:], y2[:],
                                         start=(i == 0), stop=(i == NF - 1),
                                         skip_group_check=True)
                    return psa, psb

                def ln_chain(pst):
                    psa, psb = pst
                    mu = cp2.tile([1, SQB], dt.float32, tag="mu", name="mu")
                    nc.vector.tensor_scalar_mul(mu[:], psa[0:1, :SQB],
                                                1.0 / F)
                    mu2 = cp2.tile([1, SQB], dt.float32, tag="mu2",
                                   name="mu2")
                    nc.vector.scalar_tensor_tensor(mu2[:], mu[:], 1.0, mu[:],
                                                   OP.mult, OP.mult)
                    var = cp2.tile([1, SQB], dt.float32, tag="var",
                                   name="var")
                    nc.vector.scalar_tensor_tensor(var[:], psb[0:1, :SQB],
                                                   1.0 / F, mu2[:], OP.mult,
                                                   OP.subtract)
                    # rstd = var^-0.5 on DVE (bit-trick + 1 Newton step):
                    # keeps the Activation engine exp-only, avoiding a
                    # 1.3us act-table reload between every LN and the
                    # attention exp stream. var >= ~0.9 here so eps is
                    # dropped (effect ~1e-5).
                    ti = cp2.tile([1, SQB], dt.int32, tag="ti", name="ti")
                    nc.vector.tensor_scalar(ti[:], var.bitcast(dt.int32)[:],
                                            1, None,
                                            OP.logical_shift_right)
                    y0i = cp2.tile([1, SQB], dt.int32, tag="y0i",
                                   name="y0i")
                    nc.vector.tensor_scalar(y0i[:], ti[:], -1, 0x5f3759df,
                                            OP.mult, OP.add)
                    y0 = y0i.bitcast(dt.float32)
                    r1 = cp2.tile([1, SQB], dt.float32, tag="r1", name="r1")
                    nc.vector.scalar_tensor_tensor(r1[:], y0[:], 1.0, y0[:],
                                                   OP.mult, OP.mult)
                    r2 = cp2.tile([1, SQB], dt.float32, tag="r2", name="r2")
                    nc.vector.scalar_tensor_tensor(r2[:], r1[:], -0.5,
                                                   var[:], OP.mult, OP.mult)
                    r3 = cp2.tile([1, SQB], dt.float32, tag="r3", name="r3")
                    nc.vector.tensor_scalar_add(r3[:], r2[:], 1.5)
                    rstd = cp2.tile([1, SQB], dt.float32, tag="rstd",
                                    name="rstd")
                    nc.vector.scalar_tensor_tensor(rstd[:], r3[:], 1.0,
                                                   y0[:], OP.mult, OP.mult)
                    mu_bc = cp2.tile([128, SQB], dt.float32, tag="mubc",
                                     name="mubc", bufs=1)
                    nc.gpsimd.partition_broadcast(mu_bc[:], mu[:])
                    rs_bc = cp2.tile([128, SQB], dt.float32, tag="rsbc",
                                     name="rsbc", bufs=1)
                    nc.gpsimd.partition_broadcast(rs_bc[:], rstd[:])
                    return mu_bc, rs_bc

                def ln_write(sq, y8, gt, bt, masked, wr8, bcs, irange):
                    act = False
                    cols = slice(sq * SQB, (sq + 1) * SQB)
                    colsb = slice(1 + sq * SQB, 1 + (sq + 1) * SQB)
                    mu_bc, rs_bc = bcs
                    for i in irange:
                        t1 = cp2.tile([128, SQB], dt.float32, tag="t1",
                                      name="t1")
                        nc.vector.scalar_tensor_tensor(
                            t1[:], y8[i][:], 0.0, mu_bc[:],
                            OP.add, OP.subtract)
                        t2 = cp2.tile([128, SQB], dt.float32, tag="t2",
                                      name="t2")
                        nc.vector.scalar_tensor_tensor(
                            t2[:], t1[:], gt[i][:], rs_bc[:],
                            OP.mult, OP.mult)
                        if masked:
                            # final layer: conv input = masked src
                            nc.vector.scalar_tensor_tensor(
                                SRCB[i][:, colsb], t2[:], bt[i][:],
                                mbc[:, cols], OP.add, OP.mult)
                        elif act:
                            nc.scalar.activation(SRCB[i][:, colsb], t2[:],
                                                 AF.Identity, bias=bt[i][:])
                        else:
                            nc.vector.tensor_scalar_add(
                                SRCB[i][:, colsb], t2[:], bt[i][:])
                        if wr8:
                            if act:
                                nc.scalar.activation(
                                    S8[i // 2][:, i % 2, cols], t2[:],
                                    AF.Identity, bias=bt[i][:])
                            else:
                                nc.vector.tensor_scalar_add(
                                    S8[i // 2][:, i % 2, cols], t2[:],
                                    bt[i][:])

                def c_slices(sq, layer=layer, last=last):
                    """Out-proj + LN1 + FFN + LN2 for stripe sq, split into
                    8 closures interleaved between the next stripe's heads.
                    The final stripe's closures carry across the layer
                    boundary, so layer/last are bound at creation time."""
                    cols = slice(sq * SQB, (sq + 1) * SQB)
                    colsb = slice(1 + sq * SQB, 1 + (sq + 1) * SQB)
                    y8 = [cp.tile([128, SQB], dt.float32r, tag=f"y{i}",
                                  name=f"y{i}", bufs=1) for i in range(NF)]
                    st = {}

                    def outproj(irange):
                        def f():
                            for i in irange:
                                pa = psC.tile([128, 512], dt.float32,
                                              tag="cm", name="cm")
                                for h in range(H):
                                    nc.tensor.matmul(
                                        pa[:, :SQB],
                                        wo8[h][:, i * 128:(i + 1) * 128],
                                        O8[h][:, cols],
                                        start=(h == 0), stop=(h == H - 1))
                                nc.vector.scalar_tensor_tensor(
                                    y8[i][:], pa[:, :SQB], bo_t[i][:],
                                    SRCB[i][:, colsb], OP.add, OP.add)
                        return f

                    def s_ln1_stats():
                        st["p1"] = ln_stats(y8)

                    def s_ln1_a():
                        st["bc1"] = ln_chain(st["p1"])
                        ln_write(sq, y8, g1_t, be1_t, False, True,
                                 st["bc1"], range(0, 4))

                    def s_ln1_b():
                        ln_write(sq, y8, g1_t, be1_t, False, True,
                                 st["bc1"], range(4, NF))

                    def s_ffn():
                        ph = psC.tile([128, 512], dt.float32, tag="cm",
                                      name="cm")
                        for j in range(NP):
                            nc.tensor.matmul(ph[:, :SQB], l1_4[j][:],
                                             S8[j][:, :, cols],
                                             start=(j == 0),
                                             stop=(j == NP - 1),
                                             perf_mode=DR)
                        hh = cp2.tile([128, SQB], dt.float32, tag="hh",
                                      name="hh")
                        nc.vector.tensor_scalar(hh[:], ph[:, :SQB], ISC,
                                                bl1_t[:], OP.mult, OP.add)
                        h1 = cp2.tile([128, SQB], dt.bfloat16, tag="h1",
                                      name="h1")
                        nc.vector.tensor_scalar_max(h1[:], hh[:], 0.0)
                        for i in range(NF):
                            pf = psC.tile([128, 512], dt.float32, tag="cm",
                                          name="cm")
                            nc.tensor.matmul(pf[:, :SQB],
                                             l2_1[:, i * 128:(i + 1) * 128],
                                             h1[:], start=True, stop=True)
                            nc.vector.scalar_tensor_tensor(
                                y8[i][:], pf[:, :SQB], bl2_t[i][:],
                                SRCB[i][:, colsb], OP.add, OP.add)

                    def s_ln2_stats():
                        st["p2"] = ln_stats(y8)

                    def s_ln2_w():
                        bcs = ln_chain(st["p2"])
                        ln_write(sq, y8, g2_t, be2_t, last, not last,
                                 bcs, range(NF))
                        if layer == 0:
                            for j in range(NP):
                                if sq < 2:
                                    nc.sync.dma_start(
                                        gin_a[j, :, :, cols],
                                        S8[j][:, :, cols])
                                else:
                                    nc.sync.dma_start(
                                        gin_b[j, :, :, :],
                                        S8[j][:, :, GCA:])

                    return [outproj(range(0, 4)), outproj(range(4, NF)),
                            s_ln1_stats, s_ln1_a, s_ln1_b, s_ffn,
                            s_ln2_stats, s_ln2_w]

                carry = []
                for sq in range(NSQ):
                    nxt = c_slices(sq)
                    for h in range(H):
                        b_head(sq, h)
                        if carry:
                            carry.pop(0)()
                    assert not carry
                    carry = nxt
                    if layer == 0 and sq == 0:
                        load_awqk(1)   # prefetch layer-1 A-weights
                        load_awv(1)
                    if layer == 0 and sq == 2:
                        # first gather chunk: cols [0,768) (gin writes for
                        # stripes 0/1 landed during earlier C slices)
                        gather(gin_a, gout_a)

                es_kv.close()   # KT/VT/QT dead before the carried tail

                if not last:
                    pend_c = carry   # flushed inside the next layer's A2
                else:
                    # final conv: C(2) tail interleaved with conv blocks
                    with tc.tile_pool(name="cvw", bufs=1) as cvw, \
                         tc.tile_pool(name="cv2", bufs=2) as cv2:
                        em8 = []
                        for k in range(3):
                            row = []
                            for i in range(NF):
                                et = cvw.tile([128, F], dt.bfloat16,
                                              tag=f"em{k}_{i}",
                                              name=f"em{k}_{i}")
                                nc.sync.dma_start(
                                    et[:],
                                    wemb[k, i * 128:(i + 1) * 128, :])
                                row.append(et)
                            em8.append(row)
                        bemb_t = [ldbias(f"bemb{i}",
                                         bemb[i * 128:(i + 1) * 128, :])
                                  for i in range(NF)]

                        def conv_block(tb, ofr):
                            def f():
                                base = tb * SQB
                                for of in ofr:
                                    pc = psC.tile([128, 512], dt.float32,
                                                  tag="cm", name="cm")
                                    first = True
                                    for k in range(3):
                                        for i in range(NF):
                                            nc.tensor.matmul(
                                                pc[:, :SQB],
                                                em8[k][i][:, of * 128:
                                                          (of + 1) * 128],
                                                SRCB[i][:, base + k:
                                                        base + k + SQB],
                                                start=first,
                                                stop=(k == 2 and
                                                      i == NF - 1),
                                                skip_group_check=True)
                                            first = False
                                    ob = cv2.tile([128, SQB], dt.float32,
                                                  tag="ob", name="ob")
                                    nc.scalar.activation(
                                        ob[:], pc[:, :SQB], AF.Relu,
                                        bias=bemb_t[of][:])
                                    nc.sync.dma_start(
                                        out[of * 128:(of + 1) * 128,
                                            base:base + SQB], ob[:])
                            return f

                        # conv(0) only needs LN2 cols <= 385 (stripes 0-1,
                        # already written); conv(1)/conv(2) need stripe 2.
                        seq = (carry[0:2] + [conv_block(0, range(0, 4))]
                               + carry[2:5] + [conv_block(0, range(4, NF))]
                               + carry[5:8]
                               + [conv_block(1, range(0, 4)),
                                  conv_block(1, range(4, NF)),
                                  conv_block(2, range(0, 4)),
                                  conv_block(2, range(4, NF))])
                        for f in seq:
                            f()


def _prep_host(inputs):
    """Fold weights (fp64), build per-core input maps."""
    import ml_dtypes
    bf16 = ml_dtypes.bfloat16
    f8 = dt.np(dt.float8e4)
    f32 = np.float32
    g = {k: np.asarray(v) for k, v in inputs.items()}
    x, protos = g["x"], g["prototypes"]
    in_w, in_b = g["in_w"], g["in_b"]

    wkc = np.stack([(in_w[l, F:2 * F].astype(np.float64)
                     @ g["wk"][l].astype(np.float64)).T for l in range(L)])
    bkc = np.stack([(in_w[l, F:2 * F].astype(np.float64)
                     @ g["bk"][l].astype(np.float64)
                     + in_b[l, F:2 * F]) for l in range(L)])
    wqc = np.stack([(in_w[l, :F].astype(np.float64)
                     @ g["wq"][l].astype(np.float64)).T for l in range(L)])
    bqc = np.stack([(in_w[l, :F].astype(np.float64)
                     @ g["bq"][l].astype(np.float64)
                     + in_b[l, :F]) for l in range(L)])

    def pair8(w, ncols):
        # (L, F, ncols) -> (L, NP, 128, 2*ncols) fp8 pairs, pre-scaled
        wp = (SC * w).reshape(L, NP, 2, 128, ncols).transpose(0, 1, 3, 2, 4)
        return np.ascontiguousarray(wp.reshape(L, NP, 128, 2 * ncols)
                                    ).astype(f8)

    wv_t = np.ascontiguousarray(in_w[:, 2 * F:].transpose(0, 2, 1)
                                ).astype(np.float64)
    wl1_t = np.ascontiguousarray(g["l1_w"].transpose(0, 2, 1)
                                 ).astype(np.float64)

    shared = {
        "wkc": pair8(wkc, F),
        "wqc": pair8(wqc, F),
        "wv": pair8(wv_t, F),
        "wl1": pair8(wl1_t, DFF),
        "wo": np.ascontiguousarray(
            g["out_w"].transpose(0, 2, 1)).astype(bf16),
        "wl2": np.ascontiguousarray(
            g["l2_w"].transpose(0, 2, 1)).astype(bf16),
        "wemb": np.ascontiguousarray(
            g["emb_w"].transpose(2, 1, 0)).astype(bf16),
        "bkc": bkc.astype(f32).reshape(L, F, 1),
        "bqc": bqc.astype(f32).reshape(L, F, 1),
        "bo": np.stack([
            (g["out_w"][l].astype(np.float64)
             @ in_b[l, 2 * F:].astype(np.float64) + g["out_b"][l])
            for l in range(L)]).astype(f32).reshape(L, F, 1),
        "bl1": g["l1_b"].reshape(L, DFF, 1).astype(f32),
        "bl2": g["l2_b"].reshape(L, F, 1).astype(f32),
        "g1": g["ln1_g"].reshape(L, F, 1).astype(f32),
        "be1": g["ln1_b"].reshape(L, F, 1).astype(f32),
        "g2": g["ln2_g"].reshape(L, F, 1).astype(f32),
        "be2": g["ln2_b"].reshape(L, F, 1).astype(f32),
        "bemb": g["emb_b"].reshape(F, 1).astype(f32),
    }

    def pairs_act(a):
        # (F, NLOC) -> (NP, 128, 2, NLOC)
        return np.ascontiguousarray(
            a.reshape(NP, 2, 128, NLOC).transpose(0, 2, 1, 3))

    proto_mat = protos.reshape(F, C)   # raw reshape (matches reference)
    mask0 = np.ones((1, NLOC), bf16)
    mask1 = np.zeros((1, NLOC), bf16)
    mask1[0, :T - R1] = 1.0

    in_maps = []
    for c in range(8):
        b, half = c // 2, c % 2
        src0 = np.concatenate([x[b].T, proto_mat], axis=1)   # (F, S)
        blocks = np.stack([src0[:, :NLOC], src0[:, R1:]])    # (2, F, NLOC)
        m = dict(shared)
        m["srcf8"] = np.stack([pairs_act(blocks[0]),
                               pairs_act(blocks[1])]).astype(f8)
        m["srcloc8"] = pairs_act(blocks[half]).astype(f8)
        m["srclocb"] = blocks[half].astype(bf16)
        m["maskin"] = mask0 if half == 0 else mask1
        in_maps.append(m)
    return in_maps


def run(inputs, no_cc=False, **kw):
    nc = _build_program(no_cc=no_cc)
    in_maps = _prep_host(inputs)
    res = run_bass_kernel_spmd(nc, in_maps, core_ids=list(range(8)), **kw)
    y = np.zeros((B, F, T), np.float32)
    for b in range(B):
        o0 = res.results[2 * b]["out"]
        o1 = res.results[2 * b + 1]["out"]
        y[b, :, :CONV_SPLIT] = o0[:, :CONV_SPLIT]
        y[b, :, CONV_SPLIT:] = o1[:, CONV_SPLIT - R1:T - R1]
    return y, res


def kernel(**inputs) -> np.ndarray:
    y, _ = run(inputs)
    return y


# revision 49
# speedup vs baseline: 1.5837x; 1.0215x over previous
"""Trainium2 Bass kernel for nn_DecoupledCls (RAB transformer + conv head).

Sharding: 8 cores = (batch b, sequence half). Core (b, 0) owns tokens
[0, 1152); core (b, 1) owns tokens [1096, 2248) of the S = 2248 token
sequence (T=2048 video tokens + C=200 prototype tokens). All per-token
ops run on local tokens; K/V are computed redundantly over the full S
streamed from DRAM (layer 0: host upload; layer 1: fp8 pairwise
AllGather between the two half-cores of a batch).

vs baseline: fp8e4 DoubleRow matmuls (2 k-subtiles of 128 per
instruction, 0.5 cyc/row) for the K/V/Q projections and FFN-l1
(contraction F=1024) and for the attention PV + softmax-denominator
(ones) matmuls (contraction over key-tile pairs). QK runs fp8 non-DR
(contraction dh=128). Keys are zero-padded to 2304 so all 18 key tiles
are uniform 128 wide (a masked-ones operand and zeroed V rows exclude
the 56 pad keys). exp activations merge per key-tile pair ([128,2,384]
psum -> fp8), depth-2 software pipelined against the ones/PV matmuls.
The residual stream is bf16 (SRCB) + fp8 pairs (S8); the fp32 residual
copy is dropped. Engine placement keeps the Activation engine exp-only
during attention (K/Q drains run there in phase A, LN rstd uses a DVE
bit-trick rsqrt, LN writes run on DVE, squares on Pool). Phase B
(ACT-bound attention) software-pipelines with phase C (PE-bound
out-proj/FFN/LN) by interleaving C slices of stripe sq-1 between the
heads of stripe sq; each layer's final C stripe crosses the layer
boundary and interleaves with the next layer's Q-projection (or with
the final conv), and the next layer's weights prefetch during the
previous layer's attention.

Host-side folding: the query/key 1x1 convs are fused into the MHA
in-projections (wQC = wQ_mha @ wq_conv etc., folded in fp64); fp8
weights are pre-scaled by 32 (descaled in the psum drain) to stay in
e4m3 normal range.
"""
import math
import numpy as np

import concourse.bacc as bacc
import concourse.mybir as mybir
import concourse.tile as tile
from concourse.bass_utils import run_bass_kernel_spmd

dt = mybir.dt
AF = mybir.ActivationFunctionType
OP = mybir.AluOpType
DR = mybir.MatmulPerfMode.DoubleRow

L, B, T, F, C, H, DFF = 2, 4, 2048, 1024, 200, 8, 128
S = T + C                  # 2248
SP = 2304                  # padded key count (18*128); keys [2248:2304) dead
NLOC = 1152                # local tokens per core (9*128)
R1 = S - NLOC              # 1096: start of half-1 local range
CONV_SPLIT = 1120          # conv output ownership split
SQB = 384                  # query-stripe width (3 stripes cover NLOC)
NSQ = NLOC // SQB
NF = F // 128              # 8 feature tiles
NP = NF // 2               # 4 fp8 feature-tile pairs
NT = 18                    # key tiles (128 wide, last 56 cols dead)
NJ2 = NT // 2              # 9 key-tile pairs
SC = 32.0                  # fp8 weight pre-scale (host); descaled on drain
ISC = 1.0 / SC
SCALE = float(1.0 / math.sqrt(128.0))
REPLICA_GROUPS = [[0, 1], [2, 3], [4, 5], [6, 7]]

# A1 chunks: (block, col0, width, tile0). Enumerates all 2248 keys once:
# block 0 = tokens [0,1152), block 1 cols [56,1152) = tokens [1152,2248).
# Phase a covers source cols [0,768) of both blocks (first AllGather
# chunk), phase b the rest, so layer-1 K/V can start before the second
# gather chunk lands. All chunks are key-tile aligned.
SKA = [
    (0, 0, 512, 0), (0, 512, 256, 4), (1, 56, 512, 9), (1, 568, 128, 13),
]
SKB = [
    (0, 768, 384, 6), (1, 696, 128, 14), (1, 824, 328, 15),
]
SK_BLOCKS = SKA + SKB

_CACHE = {}


def _build_program(no_cc=False):
    key = ("nc", no_cc)
    if key in _CACHE:
        return _CACHE[key]

    nc = bacc.Bacc("TRN2", target_bir_lowering=False, debug=False,
                   num_devices=8)

    def din(name, shape, dty=dt.float32):
        return nc.dram_tensor(name, list(shape), dty, kind="ExternalInput")

    srcf8 = din("srcf8", (2, NP, 128, 2, NLOC), dt.float8e4)
    srcloc8 = din("srcloc8", (NP, 128, 2, NLOC), dt.float8e4)
    srclocb = din("srclocb", (F, NLOC), dt.bfloat16)
    maskin = din("maskin", (1, NLOC), dt.bfloat16)
    wkc = din("wkc", (L, NP, 128, 2 * F), dt.float8e4)
    wv = din("wv", (L, NP, 128, 2 * F), dt.float8e4)
    wqc = din("wqc", (L, NP, 128, 2 * F), dt.float8e4)
    wl1 = din("wl1", (L, NP, 128, 2 * DFF), dt.float8e4)
    wo = din("wo", (L, F, F), dt.bfloat16)
    wl2 = din("wl2", (L, DFF, F), dt.bfloat16)
    wemb = din("wemb", (3, F, F), dt.bfloat16)
    bkc = din("bkc", (L, F, 1))
    bqc = din("bqc", (L, F, 1))
    bo = din("bo", (L, F, 1))
    bl1 = din("bl1", (L, DFF, 1))
    bl2 = din("bl2", (L, F, 1))
    g1 = din("g1", (L, F, 1))
    be1 = din("be1", (L, F, 1))
    g2 = din("g2", (L, F, 1))
    be2 = din("be2", (L, F, 1))
    bemb = din("bemb", (F, 1))
    out = nc.dram_tensor("out", [F, NLOC], dt.float32, kind="ExternalOutput")

    with tile.TileContext(nc) as tc:
        _emit(nc, tc, locals(), no_cc=no_cc)
    nc.compile()
    _CACHE[key] = nc
    return nc


def _emit(nc, tc, t, no_cc=False):
    from contextlib import ExitStack
    srcf8, srcloc8, srclocb, maskin = (t["srcf8"], t["srcloc8"],
                                       t["srclocb"], t["maskin"])
    wkc, wv, wqc, wo, wl1, wl2, wemb = (t["wkc"], t["wv"], t["wqc"], t["wo"],
                                        t["wl1"], t["wl2"], t["wemb"])
    bkc, bqc, bo, bl1, bl2 = (t["bkc"], t["bqc"], t["bo"], t["bl1"],
                              t["bl2"])
    g1, be1, g2, be2, bemb, out = (t["g1"], t["be1"], t["g2"], t["be2"],
                                   t["bemb"], t["out"])

    ctx = ExitStack()
    with ctx:
        # ---------------- hoisted pools ----------------
        cn = ctx.enter_context(tc.tile_pool(name="cn", bufs=1))
        bias = ctx.enter_context(tc.tile_pool(name="bias", bufs=1))
        res = ctx.enter_context(tc.tile_pool(name="res", bufs=1))
        dram = ctx.enter_context(tc.tile_pool(name="dram", bufs=1,
                                              space="DRAM"))
        aw = ctx.enter_context(tc.tile_pool(name="aw", bufs=1))
        bcp = ctx.enter_context(tc.tile_pool(name="bcp", bufs=1,
                                             side="right"))
        cp = ctx.enter_context(tc.tile_pool(name="cp", bufs=1))
        cp2 = ctx.enter_context(tc.tile_pool(name="cp2", bufs=2))
        psC = ctx.enter_context(tc.tile_pool(name="psC", bufs=2,
                                             space="PSUM"))

        # ---------------- resident activations (DMAs first) ----------------
        SRCB = [res.tile([128, NLOC + 2], dt.bfloat16, tag=f"srcb{i}",
                         name=f"srcb{i}") for i in range(NF)]
        S8 = [res.tile([128, 2, NLOC], dt.float8e4, tag=f"s8_{j}",
                       name=f"s8_{j}") for j in range(NP)]
        for j in range(NP):
            nc.sync.dma_start(S8[j][:], srcloc8[j])

        # A-phase weights, reloaded per layer into the same buffers
        wq4 = [aw.tile([128, 2, F], dt.float8e4, tag=f"wq{j}",
                       name=f"wq{j}") for j in range(NP)]
        wk4 = [aw.tile([128, 2, F], dt.float8e4, tag=f"wk{j}",
                       name=f"wk{j}") for j in range(NP)]
        wv4 = [aw.tile([128, 2, F], dt.float8e4, tag=f"wv{j}",
                       name=f"wv{j}") for j in range(NP)]

        def load_awq(layer):
            for j in range(NP):
                nc.sync.dma_start(wq4[j][:], wqc[layer, j])

        def load_awk(layer):
            for j in range(NP):
                nc.sync.dma_start(wk4[j][:], wkc[layer, j])

        def load_awv(layer):
            for j in range(NP):
                nc.sync.dma_start(wv4[j][:], wv[layer, j])

        # only wq up front: the first Q matmul's bias DMAs must not queue
        # behind the K/V weights on the sync ring; wk/wv follow the A2
        # emission and arrive during Q-proj compute
        load_awq(0)

        # B/C-phase weights + attention output, reused per layer
        O8 = [bcp.tile([128, NLOC], dt.bfloat16, tag=f"o8{h}",
                       name=f"o8{h}") for h in range(H)]
        wo8 = [bcp.tile([128, F], dt.bfloat16, tag=f"wo{i}",
                        name=f"wo{i}") for i in range(NF)]
        l1_4 = [bcp.tile([128, 2, DFF], dt.float8e4, tag=f"l1_{j}",
                         name=f"l1_{j}") for j in range(NP)]
        l2_1 = bcp.tile([128, F], dt.bfloat16, tag="l2", name="l2")

        def load_bc(layer):
            for i in range(NF):
                nc.sync.dma_start(wo8[i][:],
                                  wo[layer, i * 128:(i + 1) * 128, :])
            for j in range(NP):
                nc.sync.dma_start(l1_4[j][:], wl1[layer, j])
            nc.sync.dma_start(l2_1[:], wl2[layer, :, :])

        # conv-input mask (zeroes prototype tokens on the half-1 cores)
        mrow = res.tile([1, NLOC], dt.bfloat16, tag="mrow", name="mrow")
        nc.sync.dma_start(mrow[:], maskin[:])

        # ---------------- constants / pads ----------------
        # DoubleRow lhsT outer stride must be even and 16B-aligned, so the
        # ones vectors are padded to 16 columns and sliced [:, :, 0:1].
        ones8 = cn.tile([128, 2, 16], dt.float8e4)
        nc.gpsimd.memset(ones8[:], 1.0)
        ones8m = cn.tile([128, 2, 16], dt.float8e4)
        nc.gpsimd.memset(ones8m[:], 1.0)
        # zero dead-key rows [72:128) of subtile 1 (partition starts must be
        # 32-aligned, so zero [64:128) then restore [64:72))
        nc.gpsimd.memset(ones8m[64:128, 1, :], 0.0)
        nc.gpsimd.memset(ones8m[64:72, 1, :], 1.0)
        ones_f = cn.tile([128, 1], dt.float32)
        nc.gpsimd.memset(ones_f[:], 1.0)
        ones_r = cn.tile([128, 1], dt.float32r)
        nc.scalar.copy(ones_r[:], ones_f[:])
        eps_t = cn.tile([1, 1], dt.float32)
        nc.gpsimd.memset(eps_t[:], 1e-5)

        for i in range(NF):
            nc.vector.memset(SRCB[i][:, 0:1], 0.0)
            nc.vector.memset(SRCB[i][:, NLOC + 1:NLOC + 2], 0.0)
        mbc = res.tile([128, NLOC], dt.bfloat16, tag="mbc", name="mbc")
        nc.gpsimd.partition_broadcast(mbc[:], mrow[:])

        GCA = 768   # first gather chunk covers local cols [0, 768)
        gin_a = dram.tile([NP, 128, 2, GCA], dt.float8e4)
        gout_a = dram.tile([2, NP, 128, 2, GCA], dt.float8e4)
        gin_b = dram.tile([NP, 128, 2, NLOC - GCA], dt.float8e4)
        gout_b = dram.tile([2, NP, 128, 2, NLOC - GCA], dt.float8e4)

        def gather(gi, go):
            if no_cc:
                nc.sync.dma_start(go[0], gi[:])
                nc.sync.dma_start(go[1], gi[:])
            else:
                nc.gpsimd.collective_compute(
                    "AllGather", OP.bypass,
                    replica_groups=REPLICA_GROUPS,
                    ins=[gi[:]], outs=[go[:]])

        def gsrc(layer, blk, j, c0, w):
            """K/V moving-source AP for chunk cols [c0, c0+w)."""
            if layer == 0:
                return srcf8[blk, j, :, :, c0:c0 + w]
            if c0 + w <= GCA:
                return gout_a[blk, j, :, :, c0:c0 + w]
            assert c0 >= GCA
            return gout_b[blk, j, :, :, c0 - GCA:c0 - GCA + w]

        def ldbias(pool_tag, src_ap):
            b = bias.tile([src_ap.shape[0], 1], dt.float32, tag=pool_tag,
                          name=pool_tag)
            nc.sync.dma_start(b[:], src_ap)
            return b

        def interleave(primary, fillers):
            """Emit primary[i]() with fillers popped round-robin after
            each; any leftover fillers run at the end."""
            fi = 0
            for p in primary:
                p()
                if fi < len(fillers):
                    fillers[fi]()
                    fi += 1
            while fi < len(fillers):
                fillers[fi]()
                fi += 1

        pend_c = []   # C-tail slices carried across the layer boundary

        for layer in range(L):
            last = layer == L - 1
            es_kv = ExitStack()
            kv = es_kv.enter_context(
                tc.tile_pool(name=f"kv_{layer}", bufs=1))
            KT = [kv.tile([128, SP], dt.float8e4, tag=f"kt{h}",
                          name=f"kt{h}") for h in range(H)]
            VT = [kv.tile([128, 2, F], dt.float8e4, tag=f"vt{j}",
                          name=f"vt{j}") for j in range(NJ2)]
            QT = [kv.tile([128, NLOC], dt.float8e4, tag=f"qt{h}",
                          name=f"qt{h}") for h in range(H)]
            # dead-key padding: zero K columns and V rows for [2248:2304)
            for h in range(H):
                nc.vector.memset(KT[h][:, S:SP], 0.0)
            # dead V rows: zero [64:128) of the last subtile; rows [64:72)
            # are rewritten by the V drain afterwards
            nc.vector.memset(VT[NJ2 - 1][64:128, 1, :], 0.0)

            bkc_t = [ldbias(f"bkc{h}", bkc[layer, h * 128:(h + 1) * 128, :])
                     for h in range(H)]
            bqc_t = [ldbias(f"bqc{h}", bqc[layer, h * 128:(h + 1) * 128, :])
                     for h in range(H)]

            # ---- phase A (Q-proj + K/V over padded S), interleaved with
            # the previous layer's carried C tail ----
            with tc.tile_pool(name=f"a1s_{layer}", bufs=2) as a1s, \
                 tc.tile_pool(name=f"psQ_{layer}", bufs=2,
                              space="PSUM") as psQ, \
                 tc.tile_pool(name=f"psA_{layer}", bufs=2,
                              space="PSUM") as psA, \
                 tc.tile_pool(name=f"psK_{layer}", bufs=2,
                              space="PSUM") as psK:
                def a2_chunk(sq, hr):
                    def f():
                        cols = slice(sq * SQB, (sq + 1) * SQB)
                        for h in hr:
                            pq = psQ.tile([128, 512], dt.float32, tag="qm",
                                          name="qm")
                            for j in range(NP):
                                nc.tensor.matmul(
                                    pq[:, :SQB],
                                    wq4[j][:, :, h * 128:(h + 1) * 128],
                                    S8[j][:, :, cols],
                                    start=(j == 0), stop=(j == NP - 1),
                                    perf_mode=DR)
                            if h % 2 == 0:
                                nc.scalar.activation(
                                    QT[h][:, cols], pq[:, :SQB],
                                    AF.Identity, bias=bqc_t[h][:],
                                    scale=ISC)
                            else:
                                nc.vector.tensor_scalar(
                                    QT[h][:, cols], pq[:, :SQB], ISC,
                                    bqc_t[h][:], OP.mult, OP.add)
                    return f

                def a1_chunk(blk, c0, w, t0, act_all=False):
                    def f():
                        srcp = []
                        for j in range(NP):
                            st_ = a1s.tile([128, 2, 512], dt.float8e4,
                                           tag=f"sf{j}", name=f"sf{j}")
                            if layer > 0 and c0 < GCA < c0 + w:
                                # chunk straddles the gather split
                                wa = GCA - c0
                                nc.sync.dma_start(
                                    st_[:, :, :wa],
                                    gsrc(layer, blk, j, c0, wa))
                                nc.sync.dma_start(
                                    st_[:, :, wa:w],
                                    gsrc(layer, blk, j, GCA, w - wa))
                            else:
                                nc.sync.dma_start(
                                    st_[:, :, :w],
                                    gsrc(layer, blk, j, c0, w))
                            srcp.append(st_)
                        for h in range(H):
                            pk = psK.tile([128, 512], dt.float32, tag="km",
                                          name="km")
                            for j in range(NP):
                                nc.tensor.matmul(
                                    pk[:, :w],
                                    wk4[j][:, :, h * 128:(h + 1) * 128],
                                    srcp[j][:, :, :w],
                                    start=(j == 0), stop=(j == NP - 1),
                                    perf_mode=DR)
                            kslc = KT[h][:, t0 * 128:t0 * 128 + w]
                            if act_all or h % 2 == 0:
                                nc.scalar.activation(
                                    kslc, pk[:, :w], AF.Identity,
                                    bias=bkc_t[h][:], scale=ISC)
                            else:
                                nc.vector.tensor_scalar(
                                    kslc, pk[:, :w], ISC, bkc_t[h][:],
                                    OP.mult, OP.add)
                        off = 0
                        while off < w:
                            wt_ = min(128, w - off)
                            jt = t0 + off // 128
                            for ofb in range(2):
                                pv = psA.tile([128, 512], dt.float32,
                                              tag="va", name="va")
                                for j in range(NP):
                                    nc.tensor.matmul(
                                        pv[:wt_, :],
                                        srcp[j][:, :, off:off + wt_],
                                        wv4[j][:, :,
                                               ofb * 512:(ofb + 1) * 512],
                                        start=(j == 0), stop=(j == NP - 1),
                                        perf_mode=DR)
                                vslc = VT[jt // 2][:wt_, jt % 2,
                                                   ofb * 512:(ofb + 1) * 512]
                                if layer > 0:
                                    nc.scalar.activation(vslc, pv[:wt_, :],
                                                         AF.Identity,
                                                         scale=ISC)
                                else:
                                    nc.vector.tensor_scalar_mul(
                                        vslc, pv[:wt_, :], ISC)
                            off += wt_
                    return f

                a2g = [a2_chunk(sq, hr) for sq in range(NSQ)
                       for hr in (range(0, 4), range(4, H))]
                a1a = [a1_chunk(*c) for c in SKA]
                a1b = [a1_chunk(*c) for c in SKB]

                if pend_c:
                    # A2(sq<2) and phase-a K/V (gathered early) fill the
                    # PE while the carried C tail's DVE chains drain;
                    # A2(2) needs the tail's LN2, phase-b the 2nd gather.
                    s = pend_c
                    seq = (s[0:1] + a2g[0:1] + s[1:2] + a2g[1:2]
                           + s[2:3] + a2g[2:3] + s[3:4] + a2g[3:4]
                           + s[4:5] + a1a[0:1] + s[5:6] + a1a[1:2]
                           + s[6:7] + a1a[2:3] + s[7:8] + a1a[3:4])
                    for f in seq:
                        f()
                    pend_c = []
                    gather(gin_b, gout_b)
                    for f in a2g[4:6] + a1b:
                        f()
                    load_bc(1)
                else:
                    for f in a2g:
                        f()
                    load_awk(0)
                    load_awv(0)
                    for f in a1a + a1b:
                        f()

            if layer == 0:
                for i in range(NF):
                    nc.sync.dma_start(SRCB[i][:, 1:NLOC + 1],
                                      srclocb[i * 128:(i + 1) * 128, :])
                load_bc(0)

            # ---- phases B+C, software-pipelined per 384-query stripe ----
            with tc.tile_pool(name=f"b_{layer}", bufs=7) as bp, \
                 tc.tile_pool(name=f"b2_{layer}", bufs=2) as bp2, \
                 tc.tile_pool(name=f"psS_{layer}", bufs=2,
                              space="PSUM") as psS, \
                 tc.tile_pool(name=f"psO_{layer}", bufs=1,
                              space="PSUM") as psO, \
                 tc.tile_pool(name=f"psR_{layer}", bufs=1,
                              space="PSUM") as psR:

                bo_t = [ldbias(f"bo{i}", bo[layer, i * 128:(i + 1) * 128, :])
                        for i in range(NF)]
                bl1_t = ldbias("bl1", bl1[layer, :, :])
                bl2_t = [ldbias(f"bl2{i}", bl2[layer, i * 128:(i + 1) * 128, :])
                         for i in range(NF)]
                g1_t = [ldbias(f"g1{i}", g1[layer, i * 128:(i + 1) * 128, :])
                        for i in range(NF)]
                be1_t = [ldbias(f"be1{i}", be1[layer, i * 128:(i + 1) * 128, :])
                         for i in range(NF)]
                g2_t = [ldbias(f"g2{i}", g2[layer, i * 128:(i + 1) * 128, :])
                        for i in range(NF)]
                be2_t = [ldbias(f"be2{i}", be2[layer, i * 128:(i + 1) * 128, :])
                         for i in range(NF)]

                def _flush_pair(e8, j2, po, psum, h):
                    ot = ones8m if j2 == NJ2 - 1 else ones8
                    nc.tensor.matmul(
                        psum[:, :SQB], ot[:, :, 0:1], e8[:, :, :],
                        start=(j2 == 0), stop=(j2 == NJ2 - 1),
                        perf_mode=DR, skip_group_check=True)
                    nc.tensor.matmul(
                        po[:, :SQB],
                        VT[j2][:, :, h * 128:(h + 1) * 128],
                        e8[:, :, :],
                        start=(j2 == 0), stop=(j2 == NJ2 - 1),
                        perf_mode=DR, skip_group_check=True)

                def b_head(sq, h):
                    """Attention for one (stripe, head): QK, merged exp,
                    ones+PV (DoubleRow over key-tile pairs), O8 write."""
                    cols = slice(sq * SQB, (sq + 1) * SQB)
                    po = psO.tile([128, 512], dt.float32, tag="O", name="O")
                    psum = psR.tile([1, 512], dt.float32, tag="R", name="R")
                    pend = []
                    for j2 in range(NJ2):
                        pS = psS.tile([128, 2, 512], dt.float32, tag="s",
                                      name="s")
                        for sub in range(2):
                            tix = 2 * j2 + sub
                            nc.tensor.matmul(
                                pS[:, sub, :SQB],
                                KT[h][:, tix * 128:(tix + 1) * 128],
                                QT[h][:, cols], start=True, stop=True)
                        e8 = bp.tile([128, 2, SQB], dt.float8e4, tag="e8",
                                     name="e8")
                        nc.scalar.activation(e8[:], pS[:, :, :SQB],
                                             AF.Exp, scale=SCALE)
                        pend.append((e8, j2))
                        # depth-6 pipeline: PE stays 6 QK-pairs ahead of the
                        # exp-consuming ones/PV matmuls, hiding ACT latency
                        if len(pend) == 7:
                            _flush_pair(*pend.pop(0), po, psum, h)
                    while pend:
                        _flush_pair(*pend.pop(0), po, psum, h)
                    sums = bp2.tile([1, SQB], dt.float32, tag="sums",
                                    name="sums", bufs=1)
                    nc.vector.tensor_scalar_add(sums[:], psum[:, :SQB], 0.0)
                    rec = bp2.tile([1, SQB], dt.float32, tag="rec",
                                   name="rec", bufs=1)
                    nc.vector.reciprocal(rec[:], sums[:])
                    rbc = bp2.tile([128, SQB], dt.float32, tag="rbc",
                                   name="rbc", bufs=1)
                    nc.gpsimd.partition_broadcast(rbc[:], rec[:])
                    nc.vector.scalar_tensor_tensor(
                        O8[h][:, cols], po[:, :SQB], 1.0, rbc[:],
                        OP.mult, OP.mult)

                def ln_stats(y8):
                    """16 fp32r ones-matmuls into row 0 of two psC tiles."""
                    psa = psC.tile([128, 512], dt.float32, tag="cm",
                                   name="cm")
                    psb = psC.tile([128, 512], dt.float32, tag="cm",
                                   name="cm")
                    for i in range(NF):
                        y2 = cp2.tile([128, SQB], dt.float32r, tag="y2",
                                      name="y2", bufs=1)
                        nc.gpsimd.tensor_mul(y2[:], y8[i][:], y8[i][:])
                        nc.tensor.matmul(psa[0:1, :SQB], ones_r[:], y8[i][:],
                                         start=(i == 0), stop=(i == NF - 1),
                                         skip_group_check=True)
                        nc.tensor.matmul(psb[0:1, :SQB], ones_r[:], y2[:],
                                         start=(i == 0), stop=(i == NF - 1),
                                         skip_group_check=True)
                    return psa, psb

                def ln_chain(pst):
                    psa, psb = pst
                    mu = cp2.tile([1, SQB], dt.float32, tag="mu", name="mu")
                    nc.vector.tensor_scalar_mul(mu[:], psa[0:1, :SQB],
                                                1.0 / F)
                    mu2 = cp2.tile([1, SQB], dt.float32, tag="mu2",
                                   name="mu2")
                    nc.vector.scalar_tensor_tensor(mu2[:], mu[:], 1.0, mu[:],
                                                   OP.mult, OP.mult)
                    var = cp2.tile([1, SQB], dt.float32, tag="var",
                                   name="var")
                    nc.vector.scalar_tensor_tensor(var[:], psb[0:1, :SQB],
                                                   1.0 / F, mu2[:], OP.mult,
                                                   OP.subtract)
                    # rstd = var^-0.5 on DVE (bit-trick + 1 Newton step):
                    # keeps the Activation engine exp-only, avoiding a
                    # 1.3us act-table reload between every LN and the
                    # attention exp stream. var >= ~0.9 here so eps is
                    # dropped (effect ~1e-5).
                    ti = cp2.tile([1, SQB], dt.int32, tag="ti", name="ti")
                    nc.vector.tensor_scalar(ti[:], var.bitcast(dt.int32)[:],
                                            1, None,
                                            OP.logical_shift_right)
                    y0i = cp2.tile([1, SQB], dt.int32, tag="y0i",
                                   name="y0i")
                    nc.vector.tensor_scalar(y0i[:], ti[:], -1, 0x5f3759df,
                                            OP.mult, OP.add)
                    y0 = y0i.bitcast(dt.float32)
                    r1 = cp2.tile([1, SQB], dt.float32, tag="r1", name="r1")
                    nc.vector.scalar_tensor_tensor(r1[:], y0[:], 1.0, y0[:],
                                                   OP.mult, OP.mult)
                    r2 = cp2.tile([1, SQB], dt.float32, tag="r2", name="r2")
                    nc.vector.scalar_tensor_tensor(r2[:], r1[:], -0.5,
                                                   var[:], OP.mult, OP.mult)
                    r3 = cp2.tile([1, SQB], dt.float32, tag="r3", name="r3")
                    nc.vector.tensor_scalar_add(r3[:], r2[:], 1.5)
                    rstd = cp2.tile([1, SQB], dt.float32, tag="rstd",
                                    name="rstd")
                    nc.vector.scalar_tensor_tensor(rstd[:], r3[:], 1.0,
                                                   y0[:], OP.mult, OP.mult)
                    mu_bc = cp2.tile([128, SQB], dt.float32, tag="mubc",
                                     name="mubc", bufs=1)
                    nc.gpsimd.partition_broadcast(mu_bc[:], mu[:])
                    rs_bc = cp2.tile([128, SQB], dt.float32, tag="rsbc",
                                     name="rsbc", bufs=1)
                    nc.gpsimd.partition_broadcast(rs_bc[:], rstd[:])
                    return mu_bc, rs_bc

                def ln_write(sq, y8, gt, bt, masked, wr8, bcs, irange):
                    act = False
                    cols = slice(sq * SQB, (sq + 1) * SQB)
                    colsb = slice(1 + sq * SQB, 1 + (sq + 1) * SQB)
                    mu_bc, rs_bc = bcs
                    for i in irange:
                        t1 = cp2.tile([128, SQB], dt.float32, tag="t1",
                                      name="t1")
                        nc.vector.scalar_tensor_tensor(
                            t1[:], y8[i][:], 0.0, mu_bc[:],
                            OP.add, OP.subtract)
                        t2 = cp2.tile([128, SQB], dt.float32, tag="t2",
                                      name="t2")
                        nc.vector.scalar_tensor_tensor(
                            t2[:], t1[:], gt[i][:], rs_bc[:],
                            OP.mult, OP.mult)
                        if masked:
                            # final layer: conv input = masked src
                            nc.vector.scalar_tensor_tensor(
                                SRCB[i][:, colsb], t2[:], bt[i][:],
                                mbc[:, cols], OP.add, OP.mult)
                        elif act:
                            nc.scalar.activation(SRCB[i][:, colsb], t2[:],
                                                 AF.Identity, bias=bt[i][:])
                        else:
                            nc.vector.tensor_scalar_add(
                                SRCB[i][:, colsb], t2[:], bt[i][:])
                        if wr8:
                            if act:
                                nc.scalar.activation(
                                    S8[i // 2][:, i % 2, cols], t2[:],
                                    AF.Identity, bias=bt[i][:])
                            else:
                                nc.vector.tensor_scalar_add(
                                    S8[i // 2][:, i % 2, cols], t2[:],
                                    bt[i][:])

                def c_slices(sq, layer=layer, last=last):
                    """Out-proj + LN1 + FFN + LN2 for stripe sq, split into
                    8 closures interleaved between the next stripe's heads.
                    The final stripe's closures carry across the layer
                    boundary, so layer/last are bound at creation time."""
                    cols = slice(sq * SQB, (sq + 1) * SQB)
                    colsb = slice(1 + sq * SQB, 1 + (sq + 1) * SQB)
                    y8 = [cp.tile([128, SQB], dt.float32r, tag=f"y{i}",
                                  name=f"y{i}", bufs=1) for i in range(NF)]
                    st = {}

                    def outproj(irange):
                        def f():
                            for i in irange:
                                pa = psC.tile([128, 512], dt.float32,
                                              tag="cm", name="cm")
                                for h in range(H):
                                    nc.tensor.matmul(
                                        pa[:, :SQB],
                                        wo8[h][:, i * 128:(i + 1) * 128],
                                        O8[h][:, cols],
                                        start=(h == 0), stop=(h == H - 1))
                                nc.vector.scalar_tensor_tensor(
                                    y8[i][:], pa[:, :SQB], bo_t[i][:],
                                    SRCB[i][:, colsb], OP.add, OP.add)
                        return f

                    def s_ln1_stats():
                        st["p1"] = ln_stats(y8)

                    def s_ln1_a():
                        st["bc1"] = ln_chain(st["p1"])
                        ln_write(sq, y8, g1_t, be1_t, False, True,
                                 st["bc1"], range(0, 4))

                    def s_ln1_b():
                        ln_write(sq, y8, g1_t, be1_t, False, True,
                                 st["bc1"], range(4, NF))

                    def s_ffn():
                        ph = psC.tile([128, 512], dt.float32, tag="cm",
                                      name="cm")
                        for j in range(NP):
                            nc.tensor.matmul(ph[:, :SQB], l1_4[j][:],
                                             S8[j][:, :, cols],
                                             start=(j == 0),
                                             stop=(j == NP - 1),
                                             perf_mode=DR)
                        hh = cp2.tile([128, SQB], dt.float32, tag="hh",
                                      name="hh")
                        nc.vector.tensor_scalar(hh[:], ph[:, :SQB], ISC,
                                                bl1_t[:], OP.mult, OP.add)
                        h1 = cp2.tile([128, SQB], dt.bfloat16, tag="h1",
                                      name="h1")
                        nc.vector.tensor_scalar_max(h1[:], hh[:], 0.0)
                        for i in range(NF):
                            pf = psC.tile([128, 512], dt.float32, tag="cm",
                                          name="cm")
                            nc.tensor.matmul(pf[:, :SQB],
                                             l2_1[:, i * 128:(i + 1) * 128],
                                             h1[:], start=True, stop=True)
                            nc.vector.scalar_tensor_tensor(
                                y8[i][:], pf[:, :SQB], bl2_t[i][:],
                                SRCB[i][:, colsb], OP.add, OP.add)

                    def s_ln2_stats():
                        st["p2"] = ln_stats(y8)

                    def s_ln2_w():
                        bcs = ln_chain(st["p2"])
                        ln_write(sq, y8, g2_t, be2_t, last, not last,
                                 bcs, range(NF))
                        if layer == 0:
                            for j in range(NP):
                                if sq < 2:
                                    nc.sync.dma_start(
                                        gin_a[j, :, :, cols],
                                        S8[j][:, :, cols])
                                else:
                                    nc.sync.dma_start(
                                        gin_b[j, :, :, :],
                                        S8[j][:, :, GCA:])

                    return [outproj(range(0, 4)), outproj(range(4, NF)),
                            s_ln1_stats, s_ln1_a, s_ln1_b, s_ffn,
                            s_ln2_stats, s_ln2_w]

                carry = []
                for sq in range(NSQ):
                    nxt = c_slices(sq)
                    for h in range(H):
                        b_head(sq, h)
                        if carry:
                            carry.pop(0)()
                    assert not carry
                    carry = nxt
                    if layer == 0 and sq == 0:
                        load_awq(1)   # prefetch layer-1 A-weights
                        load_awk(1)
                        load_awv(1)
                    if layer == 0 and sq == 2:
                        # first gather chunk: cols [0,768) (gin writes for
                        # stripes 0/1 landed during earlier C slices)
                        gather(gin_a, gout_a)

                es_kv.close()   # KT/VT/QT dead before the carried tail

                if not last:
                    pend_c = carry   # flushed inside the next layer's A2
                else:
                    # final conv: C(2) tail interleaved with conv blocks
                    with tc.tile_pool(name="cvw", bufs=1) as cvw, \
                         tc.tile_pool(name="cv2", bufs=2) as cv2:
                        em8 = []
                        for k in range(3):
                            row = []
                            for i in range(NF):
                                et = cvw.tile([128, F], dt.bfloat16,
                                              tag=f"em{k}_{i}",
                                              name=f"em{k}_{i}")
                                nc.sync.dma_start(
                                    et[:],
                                    wemb[k, i * 128:(i + 1) * 128, :])
                                row.append(et)
                            em8.append(row)
                        bemb_t = [ldbias(f"bemb{i}",
                                         bemb[i * 128:(i + 1) * 128, :])
                                  for i in range(NF)]

                        def conv_block(tb, ofr):
                            def f():
                                base = tb * SQB
                                for of in ofr:
                                    pc = psC.tile([128, 512], dt.float32,
                                                  tag="cm", name="cm")
                                    first = True
                                    for k in range(3):
                                        for i in range(NF):
                                            nc.tensor.matmul(
                                                pc[:, :SQB],
                                                em8[k][i][:, of * 128:
                                                          (of + 1) * 128],
                                                SRCB[i][:, base + k:
                                                        base + k + SQB],
                                                start=first,
                                                stop=(k == 2 and
                                                      i == NF - 1),
                                                skip_group_check=True)
                                            first = False
                                    ob = cv2.tile([128, SQB], dt.float32,
                                                  tag="ob", name="ob")
                                    nc.scalar.activation(
                                        ob[:], pc[:, :SQB], AF.Relu,
                                        bias=bemb_t[of][:])
                                    nc.sync.dma_start(
                                        out[of * 128:(of + 1) * 128,
                                            base:base + SQB], ob[:])
                            return f

                        # conv(0) only needs LN2 cols <= 385 (stripes 0-1,
                        # already written); conv(1)/conv(2) need stripe 2.
                        seq = (carry[0:2] + [conv_block(0, range(0, 4))]
                               + carry[2:5] + [conv_block(0, range(4, NF))]
                               + carry[5:8]
                               + [conv_block(1, range(0, 4)),
                                  conv_block(1, range(4, NF)),
                                  conv_block(2, range(0, 4)),
                                  conv_block(2, range(4, NF))])
                        for f in seq:
                            f()


def _prep_host(inputs):
    """Fold weights (fp64), build per-core input maps."""
    import ml_dtypes
    bf16 = ml_dtypes.bfloat16
    f8 = dt.np(dt.float8e4)
    f32 = np.float32
    g = {k: np.asarray(v) for k, v in inputs.items()}
    x, protos = g["x"], g["prototypes"]
    in_w, in_b = g["in_w"], g["in_b"]

    wkc = np.stack([(in_w[l, F:2 * F].astype(np.float64)
                     @ g["wk"][l].astype(np.float64)).T for l in range(L)])
    bkc = np.stack([(in_w[l, F:2 * F].astype(np.float64)
                     @ g["bk"][l].astype(np.float64)
                     + in_b[l, F:2 * F]) for l in range(L)])
    wqc = np.stack([(in_w[l, :F].astype(np.float64)
                     @ g["wq"][l].astype(np.float64)).T for l in range(L)])
    bqc = np.stack([(in_w[l, :F].astype(np.float64)
                     @ g["bq"][l].astype(np.float64)
                     + in_b[l, :F]) for l in range(L)])

    def pair8(w, ncols):
        # (L, F, ncols) -> (L, NP, 128, 2*ncols) fp8 pairs, pre-scaled
        wp = (SC * w).reshape(L, NP, 2, 128, ncols).transpose(0, 1, 3, 2, 4)
        return np.ascontiguousarray(wp.reshape(L, NP, 128, 2 * ncols)
                                    ).astype(f8)

    wv_t = np.ascontiguousarray(in_w[:, 2 * F:].transpose(0, 2, 1)
                                ).astype(np.float64)
    wl1_t = np.ascontiguousarray(g["l1_w"].transpose(0, 2, 1)
                                 ).astype(np.float64)

    shared = {
        "wkc": pair8(wkc, F),
        "wqc": pair8(wqc, F),
        "wv": pair8(wv_t, F),
        "wl1": pair8(wl1_t, DFF),
        "wo": np.ascontiguousarray(
            g["out_w"].transpose(0, 2, 1)).astype(bf16),
        "wl2": np.ascontiguousarray(
            g["l2_w"].transpose(0, 2, 1)).astype(bf16),
        "wemb": np.ascontiguousarray(
            g["emb_w"].transpose(2, 1, 0)).astype(bf16),
        "bkc": bkc.astype(f32).reshape(L, F, 1),
        "bqc": bqc.astype(f32).reshape(L, F, 1),
        "bo": np.stack([
            (g["out_w"][l].astype(np.float64)
             @ in_b[l, 2 * F:].astype(np.float64) + g["out_b"][l])
            for l in range(L)]).astype(f32).reshape(L, F, 1),
        "bl1": g["l1_b"].reshape(L, DFF, 1).astype(f32),
        "bl2": g["l2_b"].reshape(L, F, 1).astype(f32),
        "g1": g["ln1_g"].reshape(L, F, 1).astype(f32),
        "be1": g["ln1_b"].reshape(L, F, 1).astype(f32),
        "g2": g["ln2_g"].reshape(L, F, 1).astype(f32),
        "be2": g["ln2_b"].reshape(L, F, 1).astype(f32),
        "bemb": g["emb_b"].reshape(F, 1).astype(f32),
    }

    def pairs_act(a):
        # (F, NLOC) -> (NP, 128, 2, NLOC)
        return np.ascontiguousarray(
            a.reshape(NP, 2, 128, NLOC).transpose(0, 2, 1, 3))

    proto_mat = protos.reshape(F, C)   # raw reshape (matches reference)
    mask0 = np.ones((1, NLOC), bf16)
    mask1 = np.zeros((1, NLOC), bf16)
    mask1[0, :T - R1] = 1.0

    in_maps = []
    for c in range(8):
        b, half = c // 2, c % 2
        src0 = np.concatenate([x[b].T, proto_mat], axis=1)   # (F, S)
        blocks = np.stack([src0[:, :NLOC], src0[:, R1:]])    # (2, F, NLOC)
        m = dict(shared)
        m["srcf8"] = np.stack([pairs_act(blocks[0]),
                               pairs_act(blocks[1])]).astype(f8)
        m["srcloc8"] = pairs_act(blocks[half]).astype(f8)
        m["srclocb"] = blocks[half].astype(bf16)
        m["maskin"] = mask0 if half == 0 else mask1
        in_maps.append(m)
    return in_maps


def run(inputs, no_cc=False, **kw):
    nc = _build_program(no_cc=no_cc)
    in_maps = _prep_host(inputs)
    res = run_bass_kernel_spmd(nc, in_maps, core_ids=list(range(8)), **kw)
    y = np.zeros((B, F, T), np.float32)
    for b in range(B):
        o0 = res.results[2 * b]["out"]
        o1 = res.results[2 * b + 1]["out"]
        y[b, :, :CONV_SPLIT] = o0[:, :CONV_SPLIT]
        y[b, :, CONV_SPLIT:] = o1[:, CONV_SPLIT - R1:T - R1]
    return y, res


def kernel(**inputs) -> np.ndarray:
    y, _ = run(inputs)
    return y
